# revision 84
# baseline (speedup 1.0000x reference)
"""Trainium2 Bass kernel for DiffusionCoordinateInitializer.

Reference computation:
    coords = einsum("bsd,cd->bsc", latent, W) + b          # [B, S, 3]
    x = noise; for t in reversed(range(T)): x = a*x + (1-a)*coords, a=(t+1)/T
which collapses (affine fixed-point iteration) to
    x = A*noise + (1-A)*(coords + b),  A = prod_{t=1..T} t/T = T!/T^T

Strategy (pure data-parallel over 8 cores, token-sharded; v11 pipeline):
  - Host folds (1-A) into W^T and A*noise + (1-A)*b into a bias tensor, so
    the device computes out^T[3, tok] = (W_eff @ latent^T) + bias^T.
  - Host pre-transposes + downcasts latent to fp8 e3m4 in a PAIR-BLOCKED
    partition-major layout [pair*128, 16 chunks, 1024 tok]: each DMA piece
    [128, 2, 1024] reads a contiguous 2 KB run per partition (2 KB DMA
    descriptors instead of the 1 KB the old [128,16,TOK] layout forced),
    1/4 the HBM traffic of fp32 (8.4 MB/core, ~24 us DMA floor). e3m4
    moving x fp16 stationary matmuls keep max-rel-err at 1.3e-2 (gate 2e-2).
  - Token-pair-major schedule: each 1024-token pair streams 8 two-chunk
    [128, 2048] pieces and runs 32 skinny matmuls (W chunk stationary
    [128, 3], moving [128, 512], 1 cyc/row) accumulating into 2 PSUM
    banks; its bias-add (DVE) + [3, 512] store drain while the next pair
    streams, so only the last pair's drain is exposed.
  - 8 warmup matmuls off a memset scratch tile ramp the PE clock from
    ~7.4 us (engines live) to the first real matmul ~11 us, so real work
    runs at full 2.4 GHz immediately; a gap here would reset the ramp
    (measured: a 1.1 us gap costs ~5 us, so warmups err on the long side).
Steady state is a razor-thin race: the PE consumes fp8 at ~307 GB/s vs
~330-370 GB/s DMA delivery, so the ~1 MB lookahead banked during warmup
(bufs=10) is the shock absorber that keeps the PE gap-free.
Measured: 45.9-48 us/core on trn2 depending on device load (exec =
~6.5 us fixed NEFF preamble + ~3.6 us warmup/data-wait + ~31 us PE stream
+ ~2.3 us drain + ~2.9 us teardown); beats the old [128,16,TOK] layout by
~0.4-0.6 us on order-balanced paired benches. Rejected by experiment:
i3-matmul noise folding (2 extra PE matmuls cost more than the tail
saves), issue-split across Sync+Scalar DGEs (out-of-order delivery vs
in-order consumption), chunk-major 2 MB pieces (PE starvation -> p-state
collapse), ascending block plans, 64-row warmup granules, SWDGE piece-0,
and partial e4m3 DoubleRow (only -0.5 us for rel-err 1.96e-2 -- too
close to the 2e-2 gate).
"""

import numpy as np
from contextlib import ExitStack

import concourse.bass as bass  # noqa: F401
import concourse.tile as tile
from concourse import bacc, mybir
from concourse.bass_utils import run_bass_kernel_spmd

N_CORES = 8
B, S, D = 4, 8192, 2048
TOK = B * S                      # 32768
TPC = TOK // N_CORES             # 4096 tokens per core
P = 128
SUPER = 512                      # tokens per PSUM bank (max psum free f32)
N_SUPER = TPC // SUPER           # 8
N_CHUNK = D // P                 # 16
F32 = mybir.dt.float32
F16 = mybir.dt.float16
F8E3 = mybir.dt.float8e3

_NC_CACHE = {}


def _build_nc_v7(lat_dt=F16, repeat=1):
    """Pre-transposed stream: latT [D, TPC] (fp16 or fp8e3m4) in DRAM,
    chunk-major accumulating matmuls into 8 PSUM banks, no transposes."""
    key = ("v7", lat_dt, repeat)
    if key in _NC_CACHE:
        return _NC_CACHE[key]

    nc = bacc.Bacc("TRN2", target_bir_lowering=False, debug=False,
                   enable_asserts=False, num_devices=N_CORES)
    latT = nc.dram_tensor("latT", [D, TPC], lat_dt, kind="ExternalInput").ap()
    # host prepacks W_eff^T chunks as [128, 16*3]: wt[p, 3k+c] = W_eff[c, 128k+p]
    wt = nc.dram_tensor("wt", [P, 3 * N_CHUNK], F16, kind="ExternalInput").ap()
    nzt = nc.dram_tensor("nzt", [3, TPC], F32, kind="ExternalInput").ap()
    out = nc.dram_tensor("out", [3, TPC], F32, kind="ExternalOutput").ap()

    with tile.TileContext(nc) as tc:
        with ExitStack() as ctx:
            const = ctx.enter_context(tc.tile_pool(name="const", bufs=1))
            lat_pool = ctx.enter_context(tc.tile_pool(name="lat", bufs=4))
            ps_pool = ctx.enter_context(tc.tile_pool(name="ps", bufs=1, space="PSUM"))
            osb_pool = ctx.enter_context(tc.tile_pool(name="osb", bufs=2))

            wt_t = const.tile([P, 3 * N_CHUNK], F16)
            nc.sync.dma_start(wt_t[:], wt[:])
            nz_t = const.tile([3, TPC], F32)
            nc.sync.dma_start(nz_t[:], nzt[:])

            for _ in range(repeat):
                pss = [ps_pool.tile([3, SUPER], F32, name=f"ps{s}", tag=f"ps{s}")
                       for s in range(N_SUPER)]
                for k in range(N_CHUNK):
                    lt = lat_pool.tile([P, TPC], F16, name="lt", tag="lt")
                    nc.sync.dma_start(lt[:], latT[k * P:(k + 1) * P, :])
                    for s in range(N_SUPER):
                        nc.tensor.matmul(
                            pss[s][:], wt_t[:, k * 3:(k + 1) * 3],
                            lt[:, s * SUPER:(s + 1) * SUPER],
                            start=(k == 0), stop=(k == N_CHUNK - 1),
                        )
                osb = osb_pool.tile([3, TPC], F32, name="osb", tag="osb")
                for s in range(N_SUPER):
                    nc.vector.tensor_add(osb[:, s * SUPER:(s + 1) * SUPER],
                                         pss[s][:], nz_t[:, s * SUPER:(s + 1) * SUPER])
                nc.sync.dma_start(out[:], osb[:])

    nc.compile()
    _NC_CACHE[key] = nc
    return nc


PIECE = 1024                     # tokens per DMA piece (2 KB/part fp16)
N_PIECE = TPC // PIECE           # 4 pieces per chunk


def _build_nc_v8(lat_dt=F16, wt_dt=F16, repeat=1):
    """Piece-granular stream + interleaved drain.

    Same math as v7 but: each 128-row d-chunk is loaded as 4 [128, 1024]
    pieces so the first matmul starts ~8 us earlier; after the last chunk,
    each super's bias-add runs on alternating Vector/Scalar engines right
    behind its stop-matmul, and its [3, 512] store issues immediately --
    the drain hides under the PE tail instead of serializing after it.
    """
    key = ("v8", lat_dt, wt_dt, repeat)
    if key in _NC_CACHE:
        return _NC_CACHE[key]

    nc = bacc.Bacc("TRN2", target_bir_lowering=False, debug=False,
                   enable_asserts=False, num_devices=N_CORES)
    latT = nc.dram_tensor("latT", [D, TPC], lat_dt, kind="ExternalInput").ap()
    wt = nc.dram_tensor("wt", [P, 3 * N_CHUNK], wt_dt, kind="ExternalInput").ap()
    nzt = nc.dram_tensor("nzt", [3, TPC], F32, kind="ExternalInput").ap()
    out = nc.dram_tensor("out", [3, TPC], F32, kind="ExternalOutput").ap()

    SPP = PIECE // SUPER  # supers per piece (2)

    with tile.TileContext(nc) as tc:
        with ExitStack() as ctx:
            const = ctx.enter_context(tc.tile_pool(name="const", bufs=1))
            lat_pool = ctx.enter_context(tc.tile_pool(name="lat", bufs=16))
            ps_pool = ctx.enter_context(tc.tile_pool(name="ps", bufs=1, space="PSUM"))
            osb_pool = ctx.enter_context(tc.tile_pool(name="osb", bufs=8))

            # consts via engine sequencers: the Sync sequencer spends the
            # first ~9 us on queue init, and a DIRECT2D issued there would
            # gate the first matmul on the weights until ~11 us.
            wt_t = const.tile([P, 3 * N_CHUNK], wt_dt)
            nc.scalar.dma_start(wt_t[:], wt[:])
            nz_t = const.tile([3, TPC], F32)
            nc.gpsimd.dma_start(nz_t[:], nzt[:])

            for _ in range(repeat):
                pss = [ps_pool.tile([3, SUPER], F32, name=f"ps{s}", tag=f"ps{s}")
                       for s in range(N_SUPER)]
                for k in range(N_CHUNK):
                    # chunk 0 in 512-token pieces so the first matmul's
                    # dependency lands ~2 us after DMA start; 1024 after
                    w = SUPER if k == 0 else PIECE
                    spp = w // SUPER
                    pieces = []
                    for p in range(TPC // w):
                        lt = lat_pool.tile([P, w], lat_dt, name="lt", tag="lt")
                        nc.sync.dma_start(
                            lt[:], latT[k * P:(k + 1) * P, p * w:(p + 1) * w])
                        pieces.append(lt)
                    for s in range(N_SUPER):
                        nc.tensor.matmul(
                            pss[s][:], wt_t[:, k * 3:(k + 1) * 3],
                            pieces[s // spp][:, (s % spp) * SUPER:
                                             (s % spp + 1) * SUPER],
                            start=(k == 0), stop=(k == N_CHUNK - 1),
                        )
                        if k == N_CHUNK - 1:
                            osb = osb_pool.tile([3, SUPER], F32,
                                                name="osb", tag="osb")
                            nc.vector.tensor_add(osb[:], pss[s][:],
                                                 nz_t[:, s * SUPER:(s + 1) * SUPER])
                            nc.scalar.dma_start(
                                out[:, s * SUPER:(s + 1) * SUPER], osb[:])

    nc.compile()
    _NC_CACHE[key] = nc
    return nc


def _build_nc_v10(lat_dt=F16, wt_dt=F16, group=2, bufs=6, repeat=1):
    """Pair-major with multi-chunk DMA pieces.

    latT3 [128, 16, TPC] host layout (partition-major) lets one DMA carry
    `group` chunks for a 1024-token pair: [128, group, 1024] -> SBUF
    [128, group*1024]. Fewer, bigger transfers = fewer PE semaphore waits
    (the ~0.2 us/piece stall tax v9 measured with 64 pieces).
    """
    key = ("v10", lat_dt, wt_dt, group, bufs, repeat, _WARMUP_MMS, _FINE_LEAD, _PAIR0_GROUP)
    if key in _NC_CACHE:
        return _NC_CACHE[key]

    nc = bacc.Bacc("TRN2", target_bir_lowering=False, debug=False,
                   enable_asserts=False, num_devices=N_CORES)
    latT3 = nc.dram_tensor("latT", [P, N_CHUNK, TPC], lat_dt,
                           kind="ExternalInput").ap()
    wt = nc.dram_tensor("wt", [P, 3 * N_CHUNK], wt_dt, kind="ExternalInput").ap()
    nzt = nc.dram_tensor("nzt", [3, TPC], F32, kind="ExternalInput").ap()
    out = nc.dram_tensor("out", [3, TPC], F32, kind="ExternalOutput").ap()

    NG = N_CHUNK // group

    with tile.TileContext(nc) as tc:
        with ExitStack() as ctx:
            const = ctx.enter_context(tc.tile_pool(name="const", bufs=1))
            lat_pool = ctx.enter_context(tc.tile_pool(name="lat", bufs=bufs))
            ps_pool = ctx.enter_context(tc.tile_pool(name="ps", bufs=1, space="PSUM"))
            osb_pool = ctx.enter_context(tc.tile_pool(name="osb", bufs=4))

            wt_t = const.tile([P, 3 * N_CHUNK], wt_dt)
            nc.scalar.dma_start(wt_t[:], wt[:])
            nz_t = const.tile([3, TPC], F32)
            nc.gpsimd.dma_start(nz_t[:], nzt[:])

            # p-state warmup: a few throwaway matmuls on a memset scratch
            # tile while the first latent pieces are still in flight, so the
            # PE clock ramps before real work starts. Results land in bank 7,
            # which that super's real group resets with start=True.
            warm = const.tile([P, SUPER], wt_dt)
            nc.vector.memset(warm[:], 1.0)

            for _ in range(repeat):
                pss = [ps_pool.tile([3, SUPER], F32, name=f"ps{s}", tag=f"ps{s}")
                       for s in range(N_SUPER)]
                # warmups wait on wt_t (~8.9 us) and end right as the first
                # latent piece lands (~11.4), carrying the clock ramp into
                # real work with no idle gap (a gap would reset the ramp;
                # starting them earlier off a self-operand measured worse)
                for _ in range(_WARMUP_MMS):
                    nc.tensor.matmul(pss[7][:], wt_t[:, 0:3], warm[:],
                                     start=True, stop=True)
                for pr in range(N_SUPER // 2):
                    # uniform piece sizes: every non-uniform variant
                    # (fine lead pieces, single-chunk pair 0) measured worse --
                    # concurrent DMAs complete fair-share, so mixed sizes delay
                    # the bulk pieces and idle gaps reset the PE clock ramp
                    grp = _PAIR0_GROUP if pr == 0 else group
                    if pr == 0 and _FINE_LEAD:
                        sizes = [1, 1] + [group] * ((N_CHUNK - 2) // group)
                    else:
                        sizes = [grp] * (N_CHUNK // grp)
                    kmap = {}
                    k0 = 0
                    for gi, sz in enumerate(sizes):
                        for i in range(sz):
                            kmap[k0 + i] = (gi, i)
                        k0 += sz
                    pieces = []
                    off = 0
                    for gi, sz in enumerate(sizes):
                        lt = lat_pool.tile([P, sz * PIECE], lat_dt,
                                           name="lt", tag="lt")
                        nc.sync.dma_start(
                            lt[:], latT3[:, off:off + sz,
                                         pr * PIECE:(pr + 1) * PIECE])
                        pieces.append(lt)
                        off += sz
                    for k in range(N_CHUNK):
                        g, i = kmap[k]
                        for j in range(2):
                            s = 2 * pr + j
                            nc.tensor.matmul(
                                pss[s][:], wt_t[:, k * 3:(k + 1) * 3],
                                pieces[g][:, i * PIECE + j * SUPER:
                                         i * PIECE + (j + 1) * SUPER],
                                start=(k == 0), stop=(k == N_CHUNK - 1),
                            )
                    for j in range(2):
                        s = 2 * pr + j
                        osb = osb_pool.tile([3, SUPER], F32, name="osb", tag="osb")
                        nc.vector.tensor_add(osb[:], pss[s][:],
                                             nz_t[:, s * SUPER:(s + 1) * SUPER])
                        eng = nc.sync if j == 0 else nc.scalar
                        eng.dma_start(
                            out[:, s * SUPER:(s + 1) * SUPER], osb[:])

    nc.compile()
    _NC_CACHE[key] = nc
    return nc


def _build_nc_v9(lat_dt=F16, wt_dt=F16, repeat=1):
    """v8 + bias-add folded into the PE and stores straight from PSUM.

    The noise/bias term enters each super's accumulation group as one extra
    matmul: stationary = I3 [3, 3], moving = nz16 [3, 512] fp16, so
    psum += I3^T @ nz = nz elementwise. No Vector/Scalar engine work at
    all; each super's [3, 512] result DMAs from PSUM as soon as its group
    stops, hiding the whole drain under the PE tail.
    """
    key = ("v9", lat_dt, wt_dt, repeat)
    if key in _NC_CACHE:
        return _NC_CACHE[key]

    nc = bacc.Bacc("TRN2", target_bir_lowering=False, debug=False,
                   enable_asserts=False, num_devices=N_CORES)
    latT = nc.dram_tensor("latT", [D, TPC], lat_dt, kind="ExternalInput").ap()
    wt = nc.dram_tensor("wt", [P, 3 * N_CHUNK], wt_dt, kind="ExternalInput").ap()
    nzt = nc.dram_tensor("nzt", [3, TPC], F32, kind="ExternalInput").ap()
    out = nc.dram_tensor("out", [3, TPC], F32, kind="ExternalOutput").ap()

    with tile.TileContext(nc) as tc:
        with ExitStack() as ctx:
            const = ctx.enter_context(tc.tile_pool(name="const", bufs=1))
            lat_pool = ctx.enter_context(tc.tile_pool(name="lat", bufs=32))
            ps_pool = ctx.enter_context(tc.tile_pool(name="ps", bufs=1, space="PSUM"))
            osb_pool = ctx.enter_context(tc.tile_pool(name="osb", bufs=4))

            wt_t = const.tile([P, 3 * N_CHUNK], wt_dt)
            nc.scalar.dma_start(wt_t[:], wt[:])
            nz_t = const.tile([3, TPC], F32)
            nc.gpsimd.dma_start(nz_t[:], nzt[:])

            for _ in range(repeat):
                pss = [ps_pool.tile([3, SUPER], F32, name=f"ps{s}", tag=f"ps{s}")
                       for s in range(N_SUPER)]
                # token-pair-major: each 1024-token pair streams all 16
                # chunks, closes its two accumulation groups, and drains
                # while the next pair streams -- no end-of-kernel drain.
                for pr in range(N_SUPER // 2):
                    pieces = []
                    for k in range(N_CHUNK):
                        lt = lat_pool.tile([P, PIECE], lat_dt, name="lt", tag="lt")
                        nc.sync.dma_start(
                            lt[:], latT[k * P:(k + 1) * P,
                                        pr * PIECE:(pr + 1) * PIECE])
                        pieces.append(lt)
                    for k in range(N_CHUNK):
                        for j in range(2):
                            s = 2 * pr + j
                            nc.tensor.matmul(
                                pss[s][:], wt_t[:, k * 3:(k + 1) * 3],
                                pieces[k][:, j * SUPER:(j + 1) * SUPER],
                                start=(k == 0), stop=(k == N_CHUNK - 1),
                            )
                    for j in range(2):
                        s = 2 * pr + j
                        osb = osb_pool.tile([3, SUPER], F32, name="osb", tag="osb")
                        nc.vector.tensor_add(osb[:], pss[s][:],
                                             nz_t[:, s * SUPER:(s + 1) * SUPER])
                        eng = nc.sync if j == 0 else nc.scalar
                        eng.dma_start(
                            out[:, s * SUPER:(s + 1) * SUPER], osb[:])

    nc.compile()
    _NC_CACHE[key] = nc
    return nc


def _build_nc_v11(lat_dt=F8E3, wt_dt=F16, group=2, bufs=10, warmups=8,
                  drain="add", store_eng="alt", issue="sync", dummy=0,
                  wt_eng="scalar", repeat=1):
    """v10 + pair-contiguous DRAM layout + decoupled warmups.

    latP [N_PAIR*128, 16, 1024] host layout: piece (pr, g0:g0+sz) reads
    latP[pr*128:(pr+1)*128, g0:g0+sz, :] whose per-partition run is
    sz KB *contiguous* in DRAM -> sz-KB DMA descriptors instead of the
    1 KB forced by the old [128, 16, TPC] layout (4x fewer descriptors at
    group=4: less DGE issue time on Sync, less per-descriptor queue tax).

    Warmup matmuls take BOTH operands from the memset scratch tile, so
    they start as soon as the Tensor sequencer is live (~6.2 us) instead
    of waiting for the weight DMA (~8.9 us): the PE clock ramp completes
    before the first real matmul, shaving the pstate tax off real work.
    """
    key = ("v11", lat_dt, wt_dt, group, bufs, warmups, drain, store_eng,
           issue, dummy, wt_eng, repeat)
    if key in _NC_CACHE:
        return _NC_CACHE[key]

    nc = bacc.Bacc("TRN2", target_bir_lowering=False, debug=False,
                   enable_asserts=False, num_devices=N_CORES)
    N_PAIR = N_SUPER // 2
    latP = nc.dram_tensor("latT", [N_PAIR * P, N_CHUNK, PIECE], lat_dt,
                          kind="ExternalInput").ap()
    wt = nc.dram_tensor("wt", [P, 3 * N_CHUNK], wt_dt, kind="ExternalInput").ap()
    nzt = nc.dram_tensor("nzt", [3, TPC], F32, kind="ExternalInput").ap()
    if drain.startswith("i3"):
        nzt16 = nc.dram_tensor("nzt16", [3, TPC], F16, kind="ExternalInput").ap()
        i3 = nc.dram_tensor("i3", [3, 3], F16, kind="ExternalInput").ap()
    out = nc.dram_tensor("out", [3, TPC], F32, kind="ExternalOutput").ap()

    NG = N_CHUNK // group

    with tile.TileContext(nc) as tc:
        with ExitStack() as ctx:
            const = ctx.enter_context(tc.tile_pool(name="const", bufs=1))
            lat_pool = ctx.enter_context(tc.tile_pool(name="lat", bufs=bufs))
            ps_pool = ctx.enter_context(tc.tile_pool(name="ps", bufs=1, space="PSUM"))
            osb_pool = ctx.enter_context(tc.tile_pool(name="osb", bufs=4))

            # memset FIRST so the warmup matmuls (gated only on it) start
            # the moment the engines come out of the init barrier
            warm = const.tile([P, SUPER], wt_dt)
            nc.vector.memset(warm[:], 0.001)

            if dummy:
                # 1-descriptor lead DMAs absorb the first-use queue-start
                # latency so piece 0's descriptors find live queues
                dmy = const.tile([1, 4], wt_dt)
                nc.sync.dma_start(dmy[:], wt[0:1, 0:4])
                dmy2 = const.tile([1, 4], wt_dt)
                nc.scalar.dma_start(dmy2[:], wt[0:1, 0:4])

            wt_t = const.tile([P, 3 * N_CHUNK], wt_dt)
            {"scalar": nc.scalar, "sync": nc.sync,
             "gp": nc.gpsimd}[wt_eng].dma_start(wt_t[:], wt[:])
            nz_t = const.tile([3, TPC], F32)
            nc.gpsimd.dma_start(nz_t[:], nzt[:])
            if drain.startswith("i3"):
                # identity [3,3] (host-supplied): psum += I3^T @ nz16
                nz16 = const.tile([3, TPC], F16)
                nc.gpsimd.dma_start(nz16[:], nzt16[:])
                i3_t = const.tile([3, 3], F16)
                nc.scalar.dma_start(i3_t[:], i3[:])

            for _ in range(repeat):
                pss = [ps_pool.tile([3, SUPER], F32, name=f"ps{s}", tag=f"ps{s}")
                       for s in range(N_SUPER)]
                # warmups gated only on the memset: start ~6.2 us, ramp the
                # PE clock while wt + piece 0 are in flight
                for _ in range(warmups):
                    nc.tensor.matmul(pss[7][:], warm[:, 0:3], warm[:],
                                     start=True, stop=True)
                n_piece = 0
                for pr in range(N_PAIR):
                    pieces = []
                    for gi in range(NG):
                        lt = lat_pool.tile([P, group * PIECE], lat_dt,
                                           name="lt", tag="lt")
                        if issue == "split":
                            eng = nc.sync if n_piece % 2 == 0 else nc.scalar
                        elif issue == "gp0" and n_piece == 0:
                            # piece 0 via the GpSimd SWDGE path: its
                            # sequencer can issue ~1.3 us before Sync's
                            # first DIRECT2D, pulling the first real
                            # matmul's dependency earlier
                            eng = nc.gpsimd
                        else:
                            eng = nc.sync
                        eng.dma_start(
                            lt[:], latP[pr * P:(pr + 1) * P,
                                        gi * group:(gi + 1) * group, :])
                        pieces.append(lt)
                        n_piece += 1
                    do_i3 = drain == "i3all" or (
                        drain == "i3last" and pr == N_PAIR - 1)
                    if do_i3:
                        for s in (2 * pr, 2 * pr + 1):
                            nc.tensor.matmul(
                                pss[s][:], i3_t[:],
                                nz16[:, s * SUPER:(s + 1) * SUPER],
                                start=True, stop=False)
                    for k in range(N_CHUNK):
                        g, i = divmod(k, group)
                        for j in range(2):
                            s = 2 * pr + j
                            nc.tensor.matmul(
                                pss[s][:], wt_t[:, k * 3:(k + 1) * 3],
                                pieces[g][:, i * PIECE + j * SUPER:
                                         i * PIECE + (j + 1) * SUPER],
                                start=(k == 0 and not do_i3),
                                stop=(k == N_CHUNK - 1),
                            )
                    for j in range(2):
                        s = 2 * pr + j
                        if store_eng == "alt":
                            eng = nc.sync if j == 0 else nc.scalar
                        elif store_eng == "gp":
                            eng = nc.gpsimd
                        elif store_eng == "gpend":
                            # gp SWDGE keeps Sync free mid-stream; the last
                            # pair's two stores go out on the now-idle
                            # Sync+Scalar HWDGEs in parallel (the single
                            # SWDGE queue would serialize them at the tail)
                            if pr == N_SUPER // 2 - 1:
                                eng = nc.sync if j == 0 else nc.scalar
                            else:
                                eng = nc.gpsimd
                        else:
                            eng = nc.scalar
                        osb = osb_pool.tile([3, SUPER], F32,
                                            name="osb", tag="osb")
                        if do_i3:
                            # noise already in PSUM via I3 matmul: pure
                            # copies, split across Vector + Scalar so the
                            # last pair's two supers drain in parallel
                            if j == 0:
                                nc.vector.tensor_copy(osb[:], pss[s][:])
                            else:
                                nc.scalar.copy(osb[:], pss[s][:])
                        else:
                            nc.vector.tensor_add(
                                osb[:], pss[s][:],
                                nz_t[:, s * SUPER:(s + 1) * SUPER])
                        eng.dma_start(
                            out[:, s * SUPER:(s + 1) * SUPER], osb[:])

    nc.compile()
    _NC_CACHE[key] = nc
    return nc


F8E4 = mybir.dt.float8e4


def _build_nc_v12(lat_dt=F8E3, wt_dt=F16, bs=4, group=2, bufs=6, warmups=8,
                  drain="i3last", dummy=0, lead=0, wtiny=0, wt_eng="scalar",
                  issue="sync", mono=0, drk=0, repeat=1):
    """Block-major: bs supers (bs*512 tokens) per block, chunk-group pieces.

    Each DMA piece is [128, group, bs*512] with a contiguous
    group*bs*512-byte run per partition: at bs=4/group=2 that is 16
    DIRECT2D issues of 512 KB (vs v11's 32+ of 256 KB) -- the Sync
    sequencer's ~0.65 us/issue serialization stops pacing the stream.
    Drains of a block's banks hide under the next block's matmuls; the
    last block's banks get the noise folded in via I3 matmuls so their
    drains are engine-parallel copies.
    lead>0: the first `lead` chunk-pieces of block 0 are single-chunk so
    the first matmul's dependency lands early.
    """
    key = ("v12", lat_dt, wt_dt, bs, group, bufs, warmups, drain, dummy,
           lead, wtiny, wt_eng, issue, mono, drk, repeat)
    if key in _NC_CACHE:
        return _NC_CACHE[key]

    nc = bacc.Bacc("TRN2", target_bir_lowering=False, debug=False,
                   enable_asserts=False, num_devices=N_CORES)
    NB = N_SUPER // bs
    TPB = bs * SUPER
    NPLAIN = N_CHUNK - 2 * drk  # chunks streamed e3m4 at 1 cyc/row
    latB = nc.dram_tensor("latT", [NB * P, NPLAIN, TPB], lat_dt,
                          kind="ExternalInput").ap()
    if drk:
        # last 2*drk chunks in e4m3 for DoubleRow (0.5 cyc/row) matmuls.
        # Weight k-tile stride must be a multiple of 16 (s3_lw dual-fp8
        # ISA restriction), so each k-tile's 3 columns sit in a 16-wide slot.
        latD = nc.dram_tensor("latD", [NB * P, 2 * drk, TPB], F8E4,
                              kind="ExternalInput").ap()
        wt4 = nc.dram_tensor("wt4", [P, 2 * drk, 16], F8E4,
                             kind="ExternalInput").ap()
    wt = nc.dram_tensor("wt", [P, 3 * N_CHUNK], wt_dt, kind="ExternalInput").ap()
    nzt = nc.dram_tensor("nzt", [3, TPC], F32, kind="ExternalInput").ap()
    if drain.startswith("i3"):
        nzt16 = nc.dram_tensor("nzt16", [3, TPC], F16, kind="ExternalInput").ap()
        i3 = nc.dram_tensor("i3", [3, 3], F16, kind="ExternalInput").ap()
    out = nc.dram_tensor("out", [3, TPC], F32, kind="ExternalOutput").ap()

    with tile.TileContext(nc) as tc:
        with ExitStack() as ctx:
            const = ctx.enter_context(tc.tile_pool(name="const", bufs=1))
            lat_pool = ctx.enter_context(tc.tile_pool(name="lat", bufs=bufs))
            ps_pool = ctx.enter_context(tc.tile_pool(name="ps", bufs=1, space="PSUM"))
            osb_pool = ctx.enter_context(tc.tile_pool(name="osb", bufs=4))

            warm = const.tile([P, SUPER], wt_dt)
            nc.vector.memset(warm[:], 0.001)

            if dummy:
                dmy = const.tile([1, 4], wt_dt)
                nc.sync.dma_start(dmy[:], wt[0:1, 0:4])

            wt_t = const.tile([P, 3 * N_CHUNK], wt_dt)
            (nc.sync if (wt_eng == "sync" or mono) else nc.scalar).dma_start(
                wt_t[:], wt[:])
            if drk:
                wt4_t = const.tile([P, 2 * drk, 16], F8E4)
                (nc.sync if mono else nc.scalar).dma_start(wt4_t[:], wt4[:])
            nz_t = const.tile([3, TPC], F32)
            (nc.sync if mono else nc.gpsimd).dma_start(nz_t[:], nzt[:])
            if drain.startswith("i3"):
                nz16 = const.tile([3, TPC], F16)
                (nc.sync if mono else nc.gpsimd).dma_start(nz16[:], nzt16[:])
                i3_t = const.tile([3, 3], F16)
                (nc.sync if mono else nc.scalar).dma_start(i3_t[:], i3[:])

            for _ in range(repeat):
                pss = [ps_pool.tile([3, SUPER], F32, name=f"ps{s}", tag=f"ps{s}")
                       for s in range(N_SUPER)]
                for _ in range(warmups):
                    nc.tensor.matmul(pss[N_SUPER - 1][:], warm[:, 0:3], warm[:],
                                     start=True, stop=True)
                # tiny tail warmups: keep the PE busy in ~27-55 ns granules
                # so a late piece 0 can't open a ramp-resetting gap, while an
                # early piece 0 only waits out the current granule
                for _ in range(wtiny):
                    nc.tensor.matmul(pss[N_SUPER - 1][:, 0:64], warm[:, 0:3],
                                     warm[:, 0:64], start=True, stop=True)
                n_piece = 0
                for b in range(NB):
                    # piece plan: optionally single-chunk leads on block 0
                    sizes = []
                    rem = NPLAIN
                    if b == 0 and lead:
                        sizes += [1] * lead
                        rem -= lead
                    sizes += [group] * (rem // group)
                    pieces = []
                    kmap = {}
                    k0 = 0
                    for gi, sz in enumerate(sizes):
                        lt = lat_pool.tile([P, sz * TPB], lat_dt,
                                           name="lt", tag="lt")
                        if issue == "split":
                            deng = nc.sync if n_piece % 2 == 0 else nc.scalar
                        else:
                            deng = nc.sync
                        n_piece += 1
                        deng.dma_start(
                            lt[:], latB[b * P:(b + 1) * P, k0:k0 + sz, :])
                        pieces.append(lt)
                        for i in range(sz):
                            kmap[k0 + i] = (gi, i)
                        k0 += sz
                    if drk:
                        drp = lat_pool.tile([P, 2 * drk, TPB], F8E4,
                                            name="drp", tag="lt")
                        nc.sync.dma_start(
                            drp[:], latD[b * P:(b + 1) * P, :, :])
                    do_i3 = drain == "i3all" or (
                        drain.startswith("i3last") and b == NB - 1)
                    if do_i3:
                        for j in range(bs):
                            s = b * bs + j
                            nc.tensor.matmul(
                                pss[s][:], i3_t[:],
                                nz16[:, s * SUPER:(s + 1) * SUPER],
                                start=True, stop=False)
                    for k in range(NPLAIN):
                        gi, i = kmap[k]
                        for j in range(bs):
                            s = b * bs + j
                            nc.tensor.matmul(
                                pss[s][:], wt_t[:, k * 3:(k + 1) * 3],
                                pieces[gi][:, i * TPB + j * SUPER:
                                           i * TPB + (j + 1) * SUPER],
                                start=(k == 0 and not do_i3),
                                stop=(k == NPLAIN - 1 and drk == 0),
                            )
                    for dr in range(drk):
                        for j in range(bs):
                            s = b * bs + j
                            nc.tensor.matmul(
                                pss[s][:], wt4_t[:, 2 * dr:2 * dr + 2, 0:3],
                                drp[:, 2 * dr:2 * dr + 2,
                                    j * SUPER:(j + 1) * SUPER],
                                start=False, stop=(dr == drk - 1),
                                perf_mode=mybir.MatmulPerfMode.DoubleRow,
                            )
                    for j in range(bs):
                        s = b * bs + j
                        eng = nc.sync if (j % 2 == 0 or mono) else nc.scalar
                        osb = osb_pool.tile([3, SUPER], F32,
                                            name="osb", tag="osb")
                        if do_i3:
                            if j % 2 == 0 or drain == "i3lastv" or mono:
                                nc.vector.tensor_copy(osb[:], pss[s][:])
                            else:
                                nc.scalar.copy(osb[:], pss[s][:])
                        else:
                            nc.vector.tensor_add(
                                osb[:], pss[s][:],
                                nz_t[:, s * SUPER:(s + 1) * SUPER])
                        eng.dma_start(
                            out[:, s * SUPER:(s + 1) * SUPER], osb[:])

    nc.compile()
    _NC_CACHE[key] = nc
    return nc


def _build_nc_v13(lat_dt=F8E3, wt_dt=F16, plan="1:2,1:2,2:2,4:2", bufs=8,
                  warmups=3, drain="i3last", wt_eng="scalar", repeat=1):
    """Variable block plan: ascending token-block sizes for an early first
    matmul, big blocks mid-stream, small-ish final block for a short drain.

    plan: comma list of supers:group per block; supers must sum to 8.
    Block b's DMA pieces are [128, group, supers*512] with contiguous
    per-partition runs (host packs per block).
    """
    key = ("v13", lat_dt, wt_dt, plan, bufs, warmups, drain, wt_eng, repeat)
    if key in _NC_CACHE:
        return _NC_CACHE[key]

    blocks = [(int(a), int(g)) for a, g in
              (p.split(":") for p in plan.split(","))]
    assert sum(b for b, _ in blocks) == N_SUPER

    nc = bacc.Bacc("TRN2", target_bir_lowering=False, debug=False,
                   enable_asserts=False, num_devices=N_CORES)
    latBs = [nc.dram_tensor(f"latT{bi}", [P, N_CHUNK, b * SUPER], lat_dt,
                            kind="ExternalInput").ap()
             for bi, (b, _) in enumerate(blocks)]
    wt = nc.dram_tensor("wt", [P, 3 * N_CHUNK], wt_dt, kind="ExternalInput").ap()
    nzt = nc.dram_tensor("nzt", [3, TPC], F32, kind="ExternalInput").ap()
    if drain.startswith("i3"):
        nzt16 = nc.dram_tensor("nzt16", [3, TPC], F16, kind="ExternalInput").ap()
        i3 = nc.dram_tensor("i3", [3, 3], F16, kind="ExternalInput").ap()
    out = nc.dram_tensor("out", [3, TPC], F32, kind="ExternalOutput").ap()

    with tile.TileContext(nc) as tc:
        with ExitStack() as ctx:
            const = ctx.enter_context(tc.tile_pool(name="const", bufs=1))
            lat_pool = ctx.enter_context(tc.tile_pool(name="lat", bufs=bufs))
            ps_pool = ctx.enter_context(tc.tile_pool(name="ps", bufs=1, space="PSUM"))
            osb_pool = ctx.enter_context(tc.tile_pool(name="osb", bufs=4))

            warm = const.tile([P, SUPER], wt_dt)
            nc.vector.memset(warm[:], 0.001)

            wt_t = const.tile([P, 3 * N_CHUNK], wt_dt)
            (nc.sync if wt_eng == "sync" else nc.scalar).dma_start(
                wt_t[:], wt[:])
            nz_t = const.tile([3, TPC], F32)
            nc.gpsimd.dma_start(nz_t[:], nzt[:])
            if drain.startswith("i3"):
                nz16 = const.tile([3, TPC], F16)
                nc.gpsimd.dma_start(nz16[:], nzt16[:])
                i3_t = const.tile([3, 3], F16)
                nc.scalar.dma_start(i3_t[:], i3[:])

            for _ in range(repeat):
                pss = [ps_pool.tile([3, SUPER], F32, name=f"ps{s}", tag=f"ps{s}")
                       for s in range(N_SUPER)]
                for _ in range(warmups):
                    nc.tensor.matmul(pss[N_SUPER - 1][:], warm[:, 0:3], warm[:],
                                     start=True, stop=True)
                s_base = 0
                for bi, (bsup, group) in enumerate(blocks):
                    tpb = bsup * SUPER
                    pieces = []
                    kmap = {}
                    for gi in range(N_CHUNK // group):
                        lt = lat_pool.tile([P, group * tpb], lat_dt,
                                           name="lt", tag="lt")
                        nc.sync.dma_start(
                            lt[:], latBs[bi][:, gi * group:(gi + 1) * group, :])
                        pieces.append(lt)
                        for i in range(group):
                            kmap[gi * group + i] = (gi, i)
                    do_i3 = drain == "i3all" or (
                        drain.startswith("i3last") and bi == len(blocks) - 1)
                    if do_i3:
                        for j in range(bsup):
                            s = s_base + j
                            nc.tensor.matmul(
                                pss[s][:], i3_t[:],
                                nz16[:, s * SUPER:(s + 1) * SUPER],
                                start=True, stop=False)
                    for k in range(N_CHUNK):
                        gi, i = kmap[k]
                        for j in range(bsup):
                            s = s_base + j
                            nc.tensor.matmul(
                                pss[s][:], wt_t[:, k * 3:(k + 1) * 3],
                                pieces[gi][:, i * tpb + j * SUPER:
                                           i * tpb + (j + 1) * SUPER],
                                start=(k == 0 and not do_i3),
                                stop=(k == N_CHUNK - 1),
                            )
                    for j in range(bsup):
                        s = s_base + j
                        eng = nc.sync if j % 2 == 0 else nc.scalar
                        osb = osb_pool.tile([3, SUPER], F32,
                                            name="osb", tag="osb")
                        if do_i3:
                            if j % 2 == 0 or drain == "i3lastv":
                                nc.vector.tensor_copy(osb[:], pss[s][:])
                            else:
                                nc.scalar.copy(osb[:], pss[s][:])
                        else:
                            nc.vector.tensor_add(
                                osb[:], pss[s][:],
                                nz_t[:, s * SUPER:(s + 1) * SUPER])
                        eng.dma_start(
                            out[:, s * SUPER:(s + 1) * SUPER], osb[:])
                    s_base += bsup

    nc.compile()
    _NC_CACHE[key] = nc
    return nc


def _coeff(T: int) -> float:
    a = 1.0
    for t in range(T):
        a *= (t + 1) / T
    return a


PIPELINE = "v11_fp8"  # "v7" | "v8_*" | "v9_*" | "v10_*" | "v11_*" | "v12_*"
_V10_GROUP = 2
_V10_BUFS = 10
_WARMUP_MMS = 6
_FINE_LEAD = False
_PAIR0_GROUP = 2
_V11_GROUP = 2
_V11_BUFS = 10
_V11_WARMUPS = 8
_V11_DRAIN = "add"    # "add" | "i3last" | "i3all"
_V11_STORE = "gpend"  # "alt" | "scalar" | "gp" | "gpend"
_V11_ISSUE = "sync"   # "sync" | "split" | "gp0"
_V11_DUMMY = 0
_V11_WTENG = "scalar"  # "scalar" | "sync" | "gp"
_V12_BS = 4
_V12_GROUP = 2
_V12_BUFS = 6
_V12_WARMUPS = 8
_V12_DRAIN = "i3last"
_V12_DUMMY = 0
_V12_LEAD = 0
_V12_WTINY = 0
_V12_WTENG = "scalar"
_V12_ISSUE = "sync"
_V12_MONO = 0
_V12_DRK = 0
_V13_PLAN = "1:2,1:2,2:2,4:2"
_V13_BUFS = 8
_V13_WARMUPS = 3
_V13_DRAIN = "i3last"
_V13_WTENG = "scalar"


def kernel(latent, W, b, noise, diffusion_steps, _trace=False, _pipeline=None):
    import ml_dtypes
    import os
    global _V11_GROUP, _V11_BUFS, _V11_WARMUPS, _V11_DRAIN, _V11_STORE
    global _V11_ISSUE, _V11_DUMMY
    _V11_GROUP = int(os.environ.get("V11_GROUP", _V11_GROUP))
    _V11_BUFS = int(os.environ.get("V11_BUFS", _V11_BUFS))
    _V11_WARMUPS = int(os.environ.get("V11_WARMUPS", _V11_WARMUPS))
    _V11_DRAIN = os.environ.get("V11_DRAIN", _V11_DRAIN)
    _V11_STORE = os.environ.get("V11_STORE", _V11_STORE)
    _V11_ISSUE = os.environ.get("V11_ISSUE", _V11_ISSUE)
    _V11_DUMMY = int(os.environ.get("V11_DUMMY", _V11_DUMMY))
    global _V11_WTENG
    _V11_WTENG = os.environ.get("V11_WTENG", _V11_WTENG)
    global _V12_BS, _V12_GROUP, _V12_BUFS, _V12_WARMUPS, _V12_DRAIN
    global _V12_DUMMY, _V12_LEAD
    _V12_BS = int(os.environ.get("V12_BS", _V12_BS))
    _V12_GROUP = int(os.environ.get("V12_GROUP", _V12_GROUP))
    _V12_BUFS = int(os.environ.get("V12_BUFS", _V12_BUFS))
    _V12_WARMUPS = int(os.environ.get("V12_WARMUPS", _V12_WARMUPS))
    _V12_DRAIN = os.environ.get("V12_DRAIN", _V12_DRAIN)
    _V12_DUMMY = int(os.environ.get("V12_DUMMY", _V12_DUMMY))
    _V12_LEAD = int(os.environ.get("V12_LEAD", _V12_LEAD))
    global _V12_WTINY, _V12_WTENG, _V12_ISSUE, _V12_MONO
    _V12_WTINY = int(os.environ.get("V12_WTINY", _V12_WTINY))
    _V12_WTENG = os.environ.get("V12_WTENG", _V12_WTENG)
    _V12_ISSUE = os.environ.get("V12_ISSUE", _V12_ISSUE)
    _V12_MONO = int(os.environ.get("V12_MONO", _V12_MONO))
    global _V12_DRK
    _V12_DRK = int(os.environ.get("V12_DRK", _V12_DRK))
    global _V13_PLAN, _V13_BUFS, _V13_WARMUPS, _V13_DRAIN, _V13_WTENG
    _V13_PLAN = os.environ.get("V13_PLAN", _V13_PLAN)
    _V13_BUFS = int(os.environ.get("V13_BUFS", _V13_BUFS))
    _V13_WARMUPS = int(os.environ.get("V13_WARMUPS", _V13_WARMUPS))
    _V13_DRAIN = os.environ.get("V13_DRAIN", _V13_DRAIN)
    _V13_WTENG = os.environ.get("V13_WTENG", _V13_WTENG)
    T = int(diffusion_steps)
    A = _coeff(T)
    pipeline = _pipeline or PIPELINE
    fp8 = pipeline.endswith("fp8")
    v9 = pipeline.startswith("v9")

    lat_flat = np.ascontiguousarray(latent.reshape(TOK, D), dtype=np.float32)
    if fp8:
        latT_h = lat_flat.astype(ml_dtypes.float8_e3m4).T  # [D, TOK] view
    else:
        latT_h = lat_flat.astype(np.float16).T
    wt_eff = np.ascontiguousarray(W.T).astype(np.float32) * np.float32(1.0 - A)
    # prepack [2048, 3] -> [128, 16*3]: chunk k (rows 128k..128k+128) at cols 3k..3k+3
    wt_packed = np.ascontiguousarray(
        wt_eff.reshape(N_CHUNK, P, 3).transpose(1, 0, 2).reshape(P, 3 * N_CHUNK)
    ).astype(np.float16)
    nz_eff = (np.float32(A) * noise.reshape(TOK, 3)
              + np.float32(1.0 - A) * b[None, :].astype(np.float32))
    nz_eff_t = np.ascontiguousarray(nz_eff.T.astype(np.float32))  # [3, TOK]

    lat_dt = mybir.dt.float8e3 if fp8 else F16
    v10 = pipeline.startswith("v10")
    v11 = pipeline.startswith("v11")
    v12 = pipeline.startswith("v12")
    v13 = pipeline.startswith("v13")
    if pipeline == "v7":
        nc = _build_nc_v7()
    elif v13:
        nc = _build_nc_v13(lat_dt=lat_dt, plan=_V13_PLAN, bufs=_V13_BUFS,
                           warmups=_V13_WARMUPS, drain=_V13_DRAIN,
                           wt_eng=_V13_WTENG)
    elif v12:
        nc = _build_nc_v12(lat_dt=lat_dt, bs=_V12_BS, group=_V12_GROUP,
                           bufs=_V12_BUFS, warmups=_V12_WARMUPS,
                           drain=_V12_DRAIN, dummy=_V12_DUMMY, lead=_V12_LEAD,
                           wtiny=_V12_WTINY, wt_eng=_V12_WTENG,
                           issue=_V12_ISSUE, mono=_V12_MONO, drk=_V12_DRK)
    elif v11:
        nc = _build_nc_v11(lat_dt=lat_dt, group=_V11_GROUP, bufs=_V11_BUFS,
                           warmups=_V11_WARMUPS, drain=_V11_DRAIN,
                           store_eng=_V11_STORE, issue=_V11_ISSUE,
                           dummy=_V11_DUMMY, wt_eng=_V11_WTENG)
    elif v10:
        nc = _build_nc_v10(lat_dt=lat_dt, group=_V10_GROUP, bufs=_V10_BUFS)
    elif v9:
        nc = _build_nc_v9(lat_dt=lat_dt)
    else:
        nc = _build_nc_v8(lat_dt=lat_dt)
    if v10:
        # [D, TOK] -> [128, 16, TOK]: partition-major chunk layout
        lat_p = np.ascontiguousarray(
            latT_h.reshape(N_CHUNK, P, TOK).transpose(1, 0, 2))
    if v11 or v12 or v13:
        # [TOK, D] fp8 -> per-core [block*128, 16, blocktok]: piece
        # (block, k-range) is contiguous per partition line in DRAM
        lat8 = latT_h.T  # the untransposed contiguous [TOK, D] downcast
        if not v13:
            blk_tok = PIECE if v11 else _V12_BS * SUPER
            n_blk = TPC // blk_tok
        drain_mode = (_V11_DRAIN if v11 else
                      _V12_DRAIN if v12 else _V13_DRAIN)
        if v13:
            v13_blocks = [int(p.split(":")[0]) for p in _V13_PLAN.split(",")]
    in_maps = []
    for c in range(N_CORES):
        if v13:
            shard = lat8[c * TPC:(c + 1) * TPC]          # [4096, 2048]
            m = {"wt": wt_packed,
                 "nzt": np.ascontiguousarray(nz_eff_t[:, c * TPC:(c + 1) * TPC])}
            t0 = 0
            for bi, bsup in enumerate(v13_blocks):
                tpb = bsup * SUPER
                sub = shard[t0:t0 + tpb]                  # [tpb, 2048]
                m[f"latT{bi}"] = np.ascontiguousarray(
                    sub.reshape(tpb, N_CHUNK, P).transpose(2, 1, 0))
                t0 += tpb
            if drain_mode.startswith("i3"):
                m["nzt16"] = m["nzt"].astype(np.float16)
                m["i3"] = np.eye(3, dtype=np.float16)
            in_maps.append(m)
            continue
        if v11 or v12:
            drk = _V12_DRK if v12 else 0
            nplain = N_CHUNK - 2 * drk
            shard = lat8[c * TPC:(c + 1) * TPC]          # [4096, 2048]
            lat_c = np.ascontiguousarray(
                shard[:, :nplain * P]
                .reshape(n_blk, blk_tok, nplain, P)
                .transpose(0, 3, 2, 1)
                .reshape(n_blk * P, nplain, blk_tok))
            m = {"latT": lat_c, "wt": wt_packed,
                 "nzt": np.ascontiguousarray(nz_eff_t[:, c * TPC:(c + 1) * TPC])}
            if drk:
                # DoubleRow chunks: e4m3 straight from the f32 source
                shard4 = lat_flat[c * TPC:(c + 1) * TPC, nplain * P:].astype(
                    ml_dtypes.float8_e4m3)
                m["latD"] = np.ascontiguousarray(
                    shard4.reshape(n_blk, blk_tok, 2 * drk, P)
                    .transpose(0, 3, 2, 1)
                    .reshape(n_blk * P, 2 * drk, blk_tok))
                wt4_h = np.zeros((P, 2 * drk, 16), dtype=ml_dtypes.float8_e4m3)
                wt4_h[:, :, 0:3] = (
                    wt_eff[nplain * P:]
                    .reshape(2 * drk, P, 3).transpose(1, 0, 2)
                    .astype(ml_dtypes.float8_e4m3))
                m["wt4"] = wt4_h
            if drain_mode.startswith("i3"):
                m["nzt16"] = m["nzt"].astype(np.float16)
                m["i3"] = np.eye(3, dtype=np.float16)
            in_maps.append(m)
            continue
        in_maps.append({
            "latT": (np.ascontiguousarray(lat_p[:, :, c * TPC:(c + 1) * TPC])
                     if v10 else
                     np.ascontiguousarray(latT_h[:, c * TPC:(c + 1) * TPC])),
            "wt": wt_packed,
            "nzt": np.ascontiguousarray(nz_eff_t[:, c * TPC:(c + 1) * TPC]),
        })
    res = run_bass_kernel_spmd(nc, in_maps, core_ids=list(range(N_CORES)),
                               trace=_trace)
    out = np.empty((TOK, 3), dtype=np.float32)
    for c in range(N_CORES):
        out[c * TPC:(c + 1) * TPC] = res.results[c]["out"].T
    if _trace:
        kernel._last_results = res
    return out.reshape(B, S, 3)



# revision 85
# speedup vs baseline: 1.1722x; 1.1722x over previous
"""Trainium2 Bass kernel for DiffusionCoordinateInitializer.

Reference computation:
    coords = einsum("bsd,cd->bsc", latent, W) + b          # [B, S, 3]
    x = noise; for t in reversed(range(T)): x = a*x + (1-a)*coords, a=(t+1)/T
which collapses (affine fixed-point iteration) to
    x = A*noise + (1-A)*(coords + b),  A = prod_{t=1..T} t/T = T!/T^T

Strategy (pure data-parallel over 8 cores, token-sharded; v11 pipeline):
  - Host folds (1-A) into W^T and A*noise + (1-A)*b into a bias tensor, so
    the device computes out^T[3, tok] = (W_eff @ latent^T) + bias^T.
  - Host pre-transposes + downcasts latent to fp8 e3m4 in a PAIR-BLOCKED
    partition-major layout [pair*128, 16 chunks, 1024 tok]: each DMA piece
    [128, 2, 1024] reads a contiguous 2 KB run per partition (2 KB DMA
    descriptors instead of the 1 KB the old [128,16,TOK] layout forced),
    1/4 the HBM traffic of fp32 (8.4 MB/core, ~24 us DMA floor). e3m4
    moving x fp16 stationary matmuls keep max-rel-err at 1.3e-2 (gate 2e-2).
  - Token-pair-major schedule: each 1024-token pair streams 8 two-chunk
    [128, 2048] pieces and runs 32 skinny matmuls (W chunk stationary
    [128, 3], moving [128, 512], 1 cyc/row) accumulating into 2 PSUM
    banks; its bias-add (DVE) + [3, 512] store drain while the next pair
    streams, so only the last pair's drain is exposed.
  - 8 warmup matmuls off a memset scratch tile ramp the PE clock from
    ~7.4 us (engines live) to the first real matmul ~11 us, so real work
    runs at full 2.4 GHz immediately; a gap here would reset the ramp
    (measured: a 1.1 us gap costs ~5 us, so warmups err on the long side).
  - Output stores ride the GpSimd SWDGE ("gpend"), keeping the Sync
    sequencer 100% dedicated to latent DIRECT2Ds mid-stream (-1.5 us,
    zero matmul gaps); the last pair's two stores switch to the by-then
    idle Sync+Scalar HWDGEs so they issue in parallel instead of
    serializing on the single SWDGE queue (-0.9 us of exposed tail).
Steady state is a razor-thin race: the PE consumes fp8 at ~307 GB/s vs
~330-370 GB/s DMA delivery, so the ~1 MB lookahead banked during warmup
(bufs=10) is the shock absorber that keeps the PE gap-free.
Measured: 45.1-46.4 us/core on trn2 (quiet device; congested windows add
2-8 us). exec = ~6.5 us fixed NEFF preamble + ~3.6 us warmup/data-wait +
~28.5 us gap-free PE stream + ~2.2 us drain/stores + ~2.9 us teardown.
Rejected by experiment: i3-matmul noise folding (2 extra PE matmuls cost
more than the tail saves), issue-split across Sync+Scalar DGEs
(out-of-order delivery vs in-order consumption), chunk-major 2 MB pieces
(PE starvation -> p-state collapse), ascending block plans, 64-row warmup
granules, SWDGE piece-0, wt via sync/gpsimd, and partial e4m3 DoubleRow
(only -0.5 us for rel-err 1.96e-2 -- too close to the 2e-2 gate).
"""

import numpy as np
from contextlib import ExitStack

import concourse.bass as bass  # noqa: F401
import concourse.tile as tile
from concourse import bacc, mybir
from concourse.bass_utils import run_bass_kernel_spmd

N_CORES = 8
B, S, D = 4, 8192, 2048
TOK = B * S                      # 32768
TPC = TOK // N_CORES             # 4096 tokens per core
P = 128
SUPER = 512                      # tokens per PSUM bank (max psum free f32)
N_SUPER = TPC // SUPER           # 8
N_CHUNK = D // P                 # 16
F32 = mybir.dt.float32
F16 = mybir.dt.float16
F8E3 = mybir.dt.float8e3

_NC_CACHE = {}


def _build_nc_v7(lat_dt=F16, repeat=1):
    """Pre-transposed stream: latT [D, TPC] (fp16 or fp8e3m4) in DRAM,
    chunk-major accumulating matmuls into 8 PSUM banks, no transposes."""
    key = ("v7", lat_dt, repeat)
    if key in _NC_CACHE:
        return _NC_CACHE[key]

    nc = bacc.Bacc("TRN2", target_bir_lowering=False, debug=False,
                   enable_asserts=False, num_devices=N_CORES)
    latT = nc.dram_tensor("latT", [D, TPC], lat_dt, kind="ExternalInput").ap()
    # host prepacks W_eff^T chunks as [128, 16*3]: wt[p, 3k+c] = W_eff[c, 128k+p]
    wt = nc.dram_tensor("wt", [P, 3 * N_CHUNK], F16, kind="ExternalInput").ap()
    nzt = nc.dram_tensor("nzt", [3, TPC], F32, kind="ExternalInput").ap()
    out = nc.dram_tensor("out", [3, TPC], F32, kind="ExternalOutput").ap()

    with tile.TileContext(nc) as tc:
        with ExitStack() as ctx:
            const = ctx.enter_context(tc.tile_pool(name="const", bufs=1))
            lat_pool = ctx.enter_context(tc.tile_pool(name="lat", bufs=4))
            ps_pool = ctx.enter_context(tc.tile_pool(name="ps", bufs=1, space="PSUM"))
            osb_pool = ctx.enter_context(tc.tile_pool(name="osb", bufs=2))

            wt_t = const.tile([P, 3 * N_CHUNK], F16)
            nc.sync.dma_start(wt_t[:], wt[:])
            nz_t = const.tile([3, TPC], F32)
            nc.sync.dma_start(nz_t[:], nzt[:])

            for _ in range(repeat):
                pss = [ps_pool.tile([3, SUPER], F32, name=f"ps{s}", tag=f"ps{s}")
                       for s in range(N_SUPER)]
                for k in range(N_CHUNK):
                    lt = lat_pool.tile([P, TPC], F16, name="lt", tag="lt")
                    nc.sync.dma_start(lt[:], latT[k * P:(k + 1) * P, :])
                    for s in range(N_SUPER):
                        nc.tensor.matmul(
                            pss[s][:], wt_t[:, k * 3:(k + 1) * 3],
                            lt[:, s * SUPER:(s + 1) * SUPER],
                            start=(k == 0), stop=(k == N_CHUNK - 1),
                        )
                osb = osb_pool.tile([3, TPC], F32, name="osb", tag="osb")
                for s in range(N_SUPER):
                    nc.vector.tensor_add(osb[:, s * SUPER:(s + 1) * SUPER],
                                         pss[s][:], nz_t[:, s * SUPER:(s + 1) * SUPER])
                nc.sync.dma_start(out[:], osb[:])

    nc.compile()
    _NC_CACHE[key] = nc
    return nc


PIECE = 1024                     # tokens per DMA piece (2 KB/part fp16)
N_PIECE = TPC // PIECE           # 4 pieces per chunk


def _build_nc_v8(lat_dt=F16, wt_dt=F16, repeat=1):
    """Piece-granular stream + interleaved drain.

    Same math as v7 but: each 128-row d-chunk is loaded as 4 [128, 1024]
    pieces so the first matmul starts ~8 us earlier; after the last chunk,
    each super's bias-add runs on alternating Vector/Scalar engines right
    behind its stop-matmul, and its [3, 512] store issues immediately --
    the drain hides under the PE tail instead of serializing after it.
    """
    key = ("v8", lat_dt, wt_dt, repeat)
    if key in _NC_CACHE:
        return _NC_CACHE[key]

    nc = bacc.Bacc("TRN2", target_bir_lowering=False, debug=False,
                   enable_asserts=False, num_devices=N_CORES)
    latT = nc.dram_tensor("latT", [D, TPC], lat_dt, kind="ExternalInput").ap()
    wt = nc.dram_tensor("wt", [P, 3 * N_CHUNK], wt_dt, kind="ExternalInput").ap()
    nzt = nc.dram_tensor("nzt", [3, TPC], F32, kind="ExternalInput").ap()
    out = nc.dram_tensor("out", [3, TPC], F32, kind="ExternalOutput").ap()

    SPP = PIECE // SUPER  # supers per piece (2)

    with tile.TileContext(nc) as tc:
        with ExitStack() as ctx:
            const = ctx.enter_context(tc.tile_pool(name="const", bufs=1))
            lat_pool = ctx.enter_context(tc.tile_pool(name="lat", bufs=16))
            ps_pool = ctx.enter_context(tc.tile_pool(name="ps", bufs=1, space="PSUM"))
            osb_pool = ctx.enter_context(tc.tile_pool(name="osb", bufs=8))

            # consts via engine sequencers: the Sync sequencer spends the
            # first ~9 us on queue init, and a DIRECT2D issued there would
            # gate the first matmul on the weights until ~11 us.
            wt_t = const.tile([P, 3 * N_CHUNK], wt_dt)
            nc.scalar.dma_start(wt_t[:], wt[:])
            nz_t = const.tile([3, TPC], F32)
            nc.gpsimd.dma_start(nz_t[:], nzt[:])

            for _ in range(repeat):
                pss = [ps_pool.tile([3, SUPER], F32, name=f"ps{s}", tag=f"ps{s}")
                       for s in range(N_SUPER)]
                for k in range(N_CHUNK):
                    # chunk 0 in 512-token pieces so the first matmul's
                    # dependency lands ~2 us after DMA start; 1024 after
                    w = SUPER if k == 0 else PIECE
                    spp = w // SUPER
                    pieces = []
                    for p in range(TPC // w):
                        lt = lat_pool.tile([P, w], lat_dt, name="lt", tag="lt")
                        nc.sync.dma_start(
                            lt[:], latT[k * P:(k + 1) * P, p * w:(p + 1) * w])
                        pieces.append(lt)
                    for s in range(N_SUPER):
                        nc.tensor.matmul(
                            pss[s][:], wt_t[:, k * 3:(k + 1) * 3],
                            pieces[s // spp][:, (s % spp) * SUPER:
                                             (s % spp + 1) * SUPER],
                            start=(k == 0), stop=(k == N_CHUNK - 1),
                        )
                        if k == N_CHUNK - 1:
                            osb = osb_pool.tile([3, SUPER], F32,
                                                name="osb", tag="osb")
                            nc.vector.tensor_add(osb[:], pss[s][:],
                                                 nz_t[:, s * SUPER:(s + 1) * SUPER])
                            nc.scalar.dma_start(
                                out[:, s * SUPER:(s + 1) * SUPER], osb[:])

    nc.compile()
    _NC_CACHE[key] = nc
    return nc


def _build_nc_v10(lat_dt=F16, wt_dt=F16, group=2, bufs=6, repeat=1):
    """Pair-major with multi-chunk DMA pieces.

    latT3 [128, 16, TPC] host layout (partition-major) lets one DMA carry
    `group` chunks for a 1024-token pair: [128, group, 1024] -> SBUF
    [128, group*1024]. Fewer, bigger transfers = fewer PE semaphore waits
    (the ~0.2 us/piece stall tax v9 measured with 64 pieces).
    """
    key = ("v10", lat_dt, wt_dt, group, bufs, repeat, _WARMUP_MMS, _FINE_LEAD, _PAIR0_GROUP)
    if key in _NC_CACHE:
        return _NC_CACHE[key]

    nc = bacc.Bacc("TRN2", target_bir_lowering=False, debug=False,
                   enable_asserts=False, num_devices=N_CORES)
    latT3 = nc.dram_tensor("latT", [P, N_CHUNK, TPC], lat_dt,
                           kind="ExternalInput").ap()
    wt = nc.dram_tensor("wt", [P, 3 * N_CHUNK], wt_dt, kind="ExternalInput").ap()
    nzt = nc.dram_tensor("nzt", [3, TPC], F32, kind="ExternalInput").ap()
    out = nc.dram_tensor("out", [3, TPC], F32, kind="ExternalOutput").ap()

    NG = N_CHUNK // group

    with tile.TileContext(nc) as tc:
        with ExitStack() as ctx:
            const = ctx.enter_context(tc.tile_pool(name="const", bufs=1))
            lat_pool = ctx.enter_context(tc.tile_pool(name="lat", bufs=bufs))
            ps_pool = ctx.enter_context(tc.tile_pool(name="ps", bufs=1, space="PSUM"))
            osb_pool = ctx.enter_context(tc.tile_pool(name="osb", bufs=4))

            wt_t = const.tile([P, 3 * N_CHUNK], wt_dt)
            nc.scalar.dma_start(wt_t[:], wt[:])
            nz_t = const.tile([3, TPC], F32)
            nc.gpsimd.dma_start(nz_t[:], nzt[:])

            # p-state warmup: a few throwaway matmuls on a memset scratch
            # tile while the first latent pieces are still in flight, so the
            # PE clock ramps before real work starts. Results land in bank 7,
            # which that super's real group resets with start=True.
            warm = const.tile([P, SUPER], wt_dt)
            nc.vector.memset(warm[:], 1.0)

            for _ in range(repeat):
                pss = [ps_pool.tile([3, SUPER], F32, name=f"ps{s}", tag=f"ps{s}")
                       for s in range(N_SUPER)]
                # warmups wait on wt_t (~8.9 us) and end right as the first
                # latent piece lands (~11.4), carrying the clock ramp into
                # real work with no idle gap (a gap would reset the ramp;
                # starting them earlier off a self-operand measured worse)
                for _ in range(_WARMUP_MMS):
                    nc.tensor.matmul(pss[7][:], wt_t[:, 0:3], warm[:],
                                     start=True, stop=True)
                for pr in range(N_SUPER // 2):
                    # uniform piece sizes: every non-uniform variant
                    # (fine lead pieces, single-chunk pair 0) measured worse --
                    # concurrent DMAs complete fair-share, so mixed sizes delay
                    # the bulk pieces and idle gaps reset the PE clock ramp
                    grp = _PAIR0_GROUP if pr == 0 else group
                    if pr == 0 and _FINE_LEAD:
                        sizes = [1, 1] + [group] * ((N_CHUNK - 2) // group)
                    else:
                        sizes = [grp] * (N_CHUNK // grp)
                    kmap = {}
                    k0 = 0
                    for gi, sz in enumerate(sizes):
                        for i in range(sz):
                            kmap[k0 + i] = (gi, i)
                        k0 += sz
                    pieces = []
                    off = 0
                    for gi, sz in enumerate(sizes):
                        lt = lat_pool.tile([P, sz * PIECE], lat_dt,
                                           name="lt", tag="lt")
                        nc.sync.dma_start(
                            lt[:], latT3[:, off:off + sz,
                                         pr * PIECE:(pr + 1) * PIECE])
                        pieces.append(lt)
                        off += sz
                    for k in range(N_CHUNK):
                        g, i = kmap[k]
                        for j in range(2):
                            s = 2 * pr + j
                            nc.tensor.matmul(
                                pss[s][:], wt_t[:, k * 3:(k + 1) * 3],
                                pieces[g][:, i * PIECE + j * SUPER:
                                         i * PIECE + (j + 1) * SUPER],
                                start=(k == 0), stop=(k == N_CHUNK - 1),
                            )
                    for j in range(2):
                        s = 2 * pr + j
                        osb = osb_pool.tile([3, SUPER], F32, name="osb", tag="osb")
                        nc.vector.tensor_add(osb[:], pss[s][:],
                                             nz_t[:, s * SUPER:(s + 1) * SUPER])
                        eng = nc.sync if j == 0 else nc.scalar
                        eng.dma_start(
                            out[:, s * SUPER:(s + 1) * SUPER], osb[:])

    nc.compile()
    _NC_CACHE[key] = nc
    return nc


def _build_nc_v9(lat_dt=F16, wt_dt=F16, repeat=1):
    """v8 + bias-add folded into the PE and stores straight from PSUM.

    The noise/bias term enters each super's accumulation group as one extra
    matmul: stationary = I3 [3, 3], moving = nz16 [3, 512] fp16, so
    psum += I3^T @ nz = nz elementwise. No Vector/Scalar engine work at
    all; each super's [3, 512] result DMAs from PSUM as soon as its group
    stops, hiding the whole drain under the PE tail.
    """
    key = ("v9", lat_dt, wt_dt, repeat)
    if key in _NC_CACHE:
        return _NC_CACHE[key]

    nc = bacc.Bacc("TRN2", target_bir_lowering=False, debug=False,
                   enable_asserts=False, num_devices=N_CORES)
    latT = nc.dram_tensor("latT", [D, TPC], lat_dt, kind="ExternalInput").ap()
    wt = nc.dram_tensor("wt", [P, 3 * N_CHUNK], wt_dt, kind="ExternalInput").ap()
    nzt = nc.dram_tensor("nzt", [3, TPC], F32, kind="ExternalInput").ap()
    out = nc.dram_tensor("out", [3, TPC], F32, kind="ExternalOutput").ap()

    with tile.TileContext(nc) as tc:
        with ExitStack() as ctx:
            const = ctx.enter_context(tc.tile_pool(name="const", bufs=1))
            lat_pool = ctx.enter_context(tc.tile_pool(name="lat", bufs=32))
            ps_pool = ctx.enter_context(tc.tile_pool(name="ps", bufs=1, space="PSUM"))
            osb_pool = ctx.enter_context(tc.tile_pool(name="osb", bufs=4))

            wt_t = const.tile([P, 3 * N_CHUNK], wt_dt)
            nc.scalar.dma_start(wt_t[:], wt[:])
            nz_t = const.tile([3, TPC], F32)
            nc.gpsimd.dma_start(nz_t[:], nzt[:])

            for _ in range(repeat):
                pss = [ps_pool.tile([3, SUPER], F32, name=f"ps{s}", tag=f"ps{s}")
                       for s in range(N_SUPER)]
                # token-pair-major: each 1024-token pair streams all 16
                # chunks, closes its two accumulation groups, and drains
                # while the next pair streams -- no end-of-kernel drain.
                for pr in range(N_SUPER // 2):
                    pieces = []
                    for k in range(N_CHUNK):
                        lt = lat_pool.tile([P, PIECE], lat_dt, name="lt", tag="lt")
                        nc.sync.dma_start(
                            lt[:], latT[k * P:(k + 1) * P,
                                        pr * PIECE:(pr + 1) * PIECE])
                        pieces.append(lt)
                    for k in range(N_CHUNK):
                        for j in range(2):
                            s = 2 * pr + j
                            nc.tensor.matmul(
                                pss[s][:], wt_t[:, k * 3:(k + 1) * 3],
                                pieces[k][:, j * SUPER:(j + 1) * SUPER],
                                start=(k == 0), stop=(k == N_CHUNK - 1),
                            )
                    for j in range(2):
                        s = 2 * pr + j
                        osb = osb_pool.tile([3, SUPER], F32, name="osb", tag="osb")
                        nc.vector.tensor_add(osb[:], pss[s][:],
                                             nz_t[:, s * SUPER:(s + 1) * SUPER])
                        eng = nc.sync if j == 0 else nc.scalar
                        eng.dma_start(
                            out[:, s * SUPER:(s + 1) * SUPER], osb[:])

    nc.compile()
    _NC_CACHE[key] = nc
    return nc


def _build_nc_v11(lat_dt=F8E3, wt_dt=F16, group=2, bufs=10, warmups=8,
                  drain="add", store_eng="alt", issue="sync", dummy=0,
                  wt_eng="scalar", repeat=1):
    """v10 + pair-contiguous DRAM layout + decoupled warmups.

    latP [N_PAIR*128, 16, 1024] host layout: piece (pr, g0:g0+sz) reads
    latP[pr*128:(pr+1)*128, g0:g0+sz, :] whose per-partition run is
    sz KB *contiguous* in DRAM -> sz-KB DMA descriptors instead of the
    1 KB forced by the old [128, 16, TPC] layout (4x fewer descriptors at
    group=4: less DGE issue time on Sync, less per-descriptor queue tax).

    Warmup matmuls take BOTH operands from the memset scratch tile, so
    they start as soon as the Tensor sequencer is live (~6.2 us) instead
    of waiting for the weight DMA (~8.9 us): the PE clock ramp completes
    before the first real matmul, shaving the pstate tax off real work.
    """
    key = ("v11", lat_dt, wt_dt, group, bufs, warmups, drain, store_eng,
           issue, dummy, wt_eng, repeat)
    if key in _NC_CACHE:
        return _NC_CACHE[key]

    nc = bacc.Bacc("TRN2", target_bir_lowering=False, debug=False,
                   enable_asserts=False, num_devices=N_CORES)
    N_PAIR = N_SUPER // 2
    latP = nc.dram_tensor("latT", [N_PAIR * P, N_CHUNK, PIECE], lat_dt,
                          kind="ExternalInput").ap()
    wt = nc.dram_tensor("wt", [P, 3 * N_CHUNK], wt_dt, kind="ExternalInput").ap()
    nzt = nc.dram_tensor("nzt", [3, TPC], F32, kind="ExternalInput").ap()
    if drain.startswith("i3"):
        nzt16 = nc.dram_tensor("nzt16", [3, TPC], F16, kind="ExternalInput").ap()
        i3 = nc.dram_tensor("i3", [3, 3], F16, kind="ExternalInput").ap()
    out = nc.dram_tensor("out", [3, TPC], F32, kind="ExternalOutput").ap()

    NG = N_CHUNK // group

    with tile.TileContext(nc) as tc:
        with ExitStack() as ctx:
            const = ctx.enter_context(tc.tile_pool(name="const", bufs=1))
            lat_pool = ctx.enter_context(tc.tile_pool(name="lat", bufs=bufs))
            ps_pool = ctx.enter_context(tc.tile_pool(name="ps", bufs=1, space="PSUM"))
            osb_pool = ctx.enter_context(tc.tile_pool(name="osb", bufs=4))

            # memset FIRST so the warmup matmuls (gated only on it) start
            # the moment the engines come out of the init barrier
            warm = const.tile([P, SUPER], wt_dt)
            nc.vector.memset(warm[:], 0.001)

            if dummy:
                # 1-descriptor lead DMAs absorb the first-use queue-start
                # latency so piece 0's descriptors find live queues
                dmy = const.tile([1, 4], wt_dt)
                nc.sync.dma_start(dmy[:], wt[0:1, 0:4])
                dmy2 = const.tile([1, 4], wt_dt)
                nc.scalar.dma_start(dmy2[:], wt[0:1, 0:4])

            wt_t = const.tile([P, 3 * N_CHUNK], wt_dt)
            {"scalar": nc.scalar, "sync": nc.sync,
             "gp": nc.gpsimd}[wt_eng].dma_start(wt_t[:], wt[:])
            nz_t = const.tile([3, TPC], F32)
            nc.gpsimd.dma_start(nz_t[:], nzt[:])
            if drain.startswith("i3"):
                # identity [3,3] (host-supplied): psum += I3^T @ nz16
                nz16 = const.tile([3, TPC], F16)
                nc.gpsimd.dma_start(nz16[:], nzt16[:])
                i3_t = const.tile([3, 3], F16)
                nc.scalar.dma_start(i3_t[:], i3[:])

            for _ in range(repeat):
                pss = [ps_pool.tile([3, SUPER], F32, name=f"ps{s}", tag=f"ps{s}")
                       for s in range(N_SUPER)]
                # warmups gated only on the memset: start ~6.2 us, ramp the
                # PE clock while wt + piece 0 are in flight
                for _ in range(warmups):
                    nc.tensor.matmul(pss[7][:], warm[:, 0:3], warm[:],
                                     start=True, stop=True)
                n_piece = 0
                for pr in range(N_PAIR):
                    pieces = []
                    for gi in range(NG):
                        lt = lat_pool.tile([P, group * PIECE], lat_dt,
                                           name="lt", tag="lt")
                        if issue == "split":
                            eng = nc.sync if n_piece % 2 == 0 else nc.scalar
                        elif issue == "gp0" and n_piece == 0:
                            # piece 0 via the GpSimd SWDGE path: its
                            # sequencer can issue ~1.3 us before Sync's
                            # first DIRECT2D, pulling the first real
                            # matmul's dependency earlier
                            eng = nc.gpsimd
                        else:
                            eng = nc.sync
                        eng.dma_start(
                            lt[:], latP[pr * P:(pr + 1) * P,
                                        gi * group:(gi + 1) * group, :])
                        pieces.append(lt)
                        n_piece += 1
                    do_i3 = drain == "i3all" or (
                        drain == "i3last" and pr == N_PAIR - 1)
                    if do_i3:
                        for s in (2 * pr, 2 * pr + 1):
                            nc.tensor.matmul(
                                pss[s][:], i3_t[:],
                                nz16[:, s * SUPER:(s + 1) * SUPER],
                                start=True, stop=False)
                    for k in range(N_CHUNK):
                        g, i = divmod(k, group)
                        for j in range(2):
                            s = 2 * pr + j
                            nc.tensor.matmul(
                                pss[s][:], wt_t[:, k * 3:(k + 1) * 3],
                                pieces[g][:, i * PIECE + j * SUPER:
                                         i * PIECE + (j + 1) * SUPER],
                                start=(k == 0 and not do_i3),
                                stop=(k == N_CHUNK - 1),
                            )
                    for j in range(2):
                        s = 2 * pr + j
                        if store_eng == "alt":
                            eng = nc.sync if j == 0 else nc.scalar
                        elif store_eng == "gp":
                            eng = nc.gpsimd
                        elif store_eng == "gpend":
                            # gp SWDGE keeps Sync free mid-stream; the last
                            # pair's two stores go out on the now-idle
                            # Sync+Scalar HWDGEs in parallel (the single
                            # SWDGE queue would serialize them at the tail)
                            if pr == N_SUPER // 2 - 1:
                                eng = nc.sync if j == 0 else nc.scalar
                            else:
                                eng = nc.gpsimd
                        else:
                            eng = nc.scalar
                        osb = osb_pool.tile([3, SUPER], F32,
                                            name="osb", tag="osb")
                        if do_i3:
                            # noise already in PSUM via I3 matmul: pure
                            # copies, split across Vector + Scalar so the
                            # last pair's two supers drain in parallel
                            if j == 0:
                                nc.vector.tensor_copy(osb[:], pss[s][:])
                            else:
                                nc.scalar.copy(osb[:], pss[s][:])
                        else:
                            nc.vector.tensor_add(
                                osb[:], pss[s][:],
                                nz_t[:, s * SUPER:(s + 1) * SUPER])
                        eng.dma_start(
                            out[:, s * SUPER:(s + 1) * SUPER], osb[:])

    nc.compile()
    _NC_CACHE[key] = nc
    return nc


F8E4 = mybir.dt.float8e4


def _build_nc_v12(lat_dt=F8E3, wt_dt=F16, bs=4, group=2, bufs=6, warmups=8,
                  drain="i3last", dummy=0, lead=0, wtiny=0, wt_eng="scalar",
                  issue="sync", mono=0, drk=0, repeat=1):
    """Block-major: bs supers (bs*512 tokens) per block, chunk-group pieces.

    Each DMA piece is [128, group, bs*512] with a contiguous
    group*bs*512-byte run per partition: at bs=4/group=2 that is 16
    DIRECT2D issues of 512 KB (vs v11's 32+ of 256 KB) -- the Sync
    sequencer's ~0.65 us/issue serialization stops pacing the stream.
    Drains of a block's banks hide under the next block's matmuls; the
    last block's banks get the noise folded in via I3 matmuls so their
    drains are engine-parallel copies.
    lead>0: the first `lead` chunk-pieces of block 0 are single-chunk so
    the first matmul's dependency lands early.
    """
    key = ("v12", lat_dt, wt_dt, bs, group, bufs, warmups, drain, dummy,
           lead, wtiny, wt_eng, issue, mono, drk, repeat)
    if key in _NC_CACHE:
        return _NC_CACHE[key]

    nc = bacc.Bacc("TRN2", target_bir_lowering=False, debug=False,
                   enable_asserts=False, num_devices=N_CORES)
    NB = N_SUPER // bs
    TPB = bs * SUPER
    NPLAIN = N_CHUNK - 2 * drk  # chunks streamed e3m4 at 1 cyc/row
    latB = nc.dram_tensor("latT", [NB * P, NPLAIN, TPB], lat_dt,
                          kind="ExternalInput").ap()
    if drk:
        # last 2*drk chunks in e4m3 for DoubleRow (0.5 cyc/row) matmuls.
        # Weight k-tile stride must be a multiple of 16 (s3_lw dual-fp8
        # ISA restriction), so each k-tile's 3 columns sit in a 16-wide slot.
        latD = nc.dram_tensor("latD", [NB * P, 2 * drk, TPB], F8E4,
                              kind="ExternalInput").ap()
        wt4 = nc.dram_tensor("wt4", [P, 2 * drk, 16], F8E4,
                             kind="ExternalInput").ap()
    wt = nc.dram_tensor("wt", [P, 3 * N_CHUNK], wt_dt, kind="ExternalInput").ap()
    nzt = nc.dram_tensor("nzt", [3, TPC], F32, kind="ExternalInput").ap()
    if drain.startswith("i3"):
        nzt16 = nc.dram_tensor("nzt16", [3, TPC], F16, kind="ExternalInput").ap()
        i3 = nc.dram_tensor("i3", [3, 3], F16, kind="ExternalInput").ap()
    out = nc.dram_tensor("out", [3, TPC], F32, kind="ExternalOutput").ap()

    with tile.TileContext(nc) as tc:
        with ExitStack() as ctx:
            const = ctx.enter_context(tc.tile_pool(name="const", bufs=1))
            lat_pool = ctx.enter_context(tc.tile_pool(name="lat", bufs=bufs))
            ps_pool = ctx.enter_context(tc.tile_pool(name="ps", bufs=1, space="PSUM"))
            osb_pool = ctx.enter_context(tc.tile_pool(name="osb", bufs=4))

            warm = const.tile([P, SUPER], wt_dt)
            nc.vector.memset(warm[:], 0.001)

            if dummy:
                dmy = const.tile([1, 4], wt_dt)
                nc.sync.dma_start(dmy[:], wt[0:1, 0:4])

            wt_t = const.tile([P, 3 * N_CHUNK], wt_dt)
            (nc.sync if (wt_eng == "sync" or mono) else nc.scalar).dma_start(
                wt_t[:], wt[:])
            if drk:
                wt4_t = const.tile([P, 2 * drk, 16], F8E4)
                (nc.sync if mono else nc.scalar).dma_start(wt4_t[:], wt4[:])
            nz_t = const.tile([3, TPC], F32)
            (nc.sync if mono else nc.gpsimd).dma_start(nz_t[:], nzt[:])
            if drain.startswith("i3"):
                nz16 = const.tile([3, TPC], F16)
                (nc.sync if mono else nc.gpsimd).dma_start(nz16[:], nzt16[:])
                i3_t = const.tile([3, 3], F16)
                (nc.sync if mono else nc.scalar).dma_start(i3_t[:], i3[:])

            for _ in range(repeat):
                pss = [ps_pool.tile([3, SUPER], F32, name=f"ps{s}", tag=f"ps{s}")
                       for s in range(N_SUPER)]
                for _ in range(warmups):
                    nc.tensor.matmul(pss[N_SUPER - 1][:], warm[:, 0:3], warm[:],
                                     start=True, stop=True)
                # tiny tail warmups: keep the PE busy in ~27-55 ns granules
                # so a late piece 0 can't open a ramp-resetting gap, while an
                # early piece 0 only waits out the current granule
                for _ in range(wtiny):
                    nc.tensor.matmul(pss[N_SUPER - 1][:, 0:64], warm[:, 0:3],
                                     warm[:, 0:64], start=True, stop=True)
                n_piece = 0
                for b in range(NB):
                    # piece plan: optionally single-chunk leads on block 0
                    sizes = []
                    rem = NPLAIN
                    if b == 0 and lead:
                        sizes += [1] * lead
                        rem -= lead
                    sizes += [group] * (rem // group)
                    pieces = []
                    kmap = {}
                    k0 = 0
                    for gi, sz in enumerate(sizes):
                        lt = lat_pool.tile([P, sz * TPB], lat_dt,
                                           name="lt", tag="lt")
                        if issue == "split":
                            deng = nc.sync if n_piece % 2 == 0 else nc.scalar
                        else:
                            deng = nc.sync
                        n_piece += 1
                        deng.dma_start(
                            lt[:], latB[b * P:(b + 1) * P, k0:k0 + sz, :])
                        pieces.append(lt)
                        for i in range(sz):
                            kmap[k0 + i] = (gi, i)
                        k0 += sz
                    if drk:
                        drp = lat_pool.tile([P, 2 * drk, TPB], F8E4,
                                            name="drp", tag="lt")
                        nc.sync.dma_start(
                            drp[:], latD[b * P:(b + 1) * P, :, :])
                    do_i3 = drain == "i3all" or (
                        drain.startswith("i3last") and b == NB - 1)
                    if do_i3:
                        for j in range(bs):
                            s = b * bs + j
                            nc.tensor.matmul(
                                pss[s][:], i3_t[:],
                                nz16[:, s * SUPER:(s + 1) * SUPER],
                                start=True, stop=False)
                    for k in range(NPLAIN):
                        gi, i = kmap[k]
                        for j in range(bs):
                            s = b * bs + j
                            nc.tensor.matmul(
                                pss[s][:], wt_t[:, k * 3:(k + 1) * 3],
                                pieces[gi][:, i * TPB + j * SUPER:
                                           i * TPB + (j + 1) * SUPER],
                                start=(k == 0 and not do_i3),
                                stop=(k == NPLAIN - 1 and drk == 0),
                            )
                    for dr in range(drk):
                        for j in range(bs):
                            s = b * bs + j
                            nc.tensor.matmul(
                                pss[s][:], wt4_t[:, 2 * dr:2 * dr + 2, 0:3],
                                drp[:, 2 * dr:2 * dr + 2,
                                    j * SUPER:(j + 1) * SUPER],
                                start=False, stop=(dr == drk - 1),
                                perf_mode=mybir.MatmulPerfMode.DoubleRow,
                            )
                    for j in range(bs):
                        s = b * bs + j
                        eng = nc.sync if (j % 2 == 0 or mono) else nc.scalar
                        osb = osb_pool.tile([3, SUPER], F32,
                                            name="osb", tag="osb")
                        if do_i3:
                            if j % 2 == 0 or drain == "i3lastv" or mono:
                                nc.vector.tensor_copy(osb[:], pss[s][:])
                            else:
                                nc.scalar.copy(osb[:], pss[s][:])
                        else:
                            nc.vector.tensor_add(
                                osb[:], pss[s][:],
                                nz_t[:, s * SUPER:(s + 1) * SUPER])
                        eng.dma_start(
                            out[:, s * SUPER:(s + 1) * SUPER], osb[:])

    nc.compile()
    _NC_CACHE[key] = nc
    return nc


def _build_nc_v13(lat_dt=F8E3, wt_dt=F16, plan="1:2,1:2,2:2,4:2", bufs=8,
                  warmups=3, drain="i3last", wt_eng="scalar", repeat=1):
    """Variable block plan: ascending token-block sizes for an early first
    matmul, big blocks mid-stream, small-ish final block for a short drain.

    plan: comma list of supers:group per block; supers must sum to 8.
    Block b's DMA pieces are [128, group, supers*512] with contiguous
    per-partition runs (host packs per block).
    """
    key = ("v13", lat_dt, wt_dt, plan, bufs, warmups, drain, wt_eng, repeat)
    if key in _NC_CACHE:
        return _NC_CACHE[key]

    blocks = [(int(a), int(g)) for a, g in
              (p.split(":") for p in plan.split(","))]
    assert sum(b for b, _ in blocks) == N_SUPER

    nc = bacc.Bacc("TRN2", target_bir_lowering=False, debug=False,
                   enable_asserts=False, num_devices=N_CORES)
    latBs = [nc.dram_tensor(f"latT{bi}", [P, N_CHUNK, b * SUPER], lat_dt,
                            kind="ExternalInput").ap()
             for bi, (b, _) in enumerate(blocks)]
    wt = nc.dram_tensor("wt", [P, 3 * N_CHUNK], wt_dt, kind="ExternalInput").ap()
    nzt = nc.dram_tensor("nzt", [3, TPC], F32, kind="ExternalInput").ap()
    if drain.startswith("i3"):
        nzt16 = nc.dram_tensor("nzt16", [3, TPC], F16, kind="ExternalInput").ap()
        i3 = nc.dram_tensor("i3", [3, 3], F16, kind="ExternalInput").ap()
    out = nc.dram_tensor("out", [3, TPC], F32, kind="ExternalOutput").ap()

    with tile.TileContext(nc) as tc:
        with ExitStack() as ctx:
            const = ctx.enter_context(tc.tile_pool(name="const", bufs=1))
            lat_pool = ctx.enter_context(tc.tile_pool(name="lat", bufs=bufs))
            ps_pool = ctx.enter_context(tc.tile_pool(name="ps", bufs=1, space="PSUM"))
            osb_pool = ctx.enter_context(tc.tile_pool(name="osb", bufs=4))

            warm = const.tile([P, SUPER], wt_dt)
            nc.vector.memset(warm[:], 0.001)

            wt_t = const.tile([P, 3 * N_CHUNK], wt_dt)
            (nc.sync if wt_eng == "sync" else nc.scalar).dma_start(
                wt_t[:], wt[:])
            nz_t = const.tile([3, TPC], F32)
            nc.gpsimd.dma_start(nz_t[:], nzt[:])
            if drain.startswith("i3"):
                nz16 = const.tile([3, TPC], F16)
                nc.gpsimd.dma_start(nz16[:], nzt16[:])
                i3_t = const.tile([3, 3], F16)
                nc.scalar.dma_start(i3_t[:], i3[:])

            for _ in range(repeat):
                pss = [ps_pool.tile([3, SUPER], F32, name=f"ps{s}", tag=f"ps{s}")
                       for s in range(N_SUPER)]
                for _ in range(warmups):
                    nc.tensor.matmul(pss[N_SUPER - 1][:], warm[:, 0:3], warm[:],
                                     start=True, stop=True)
                s_base = 0
                for bi, (bsup, group) in enumerate(blocks):
                    tpb = bsup * SUPER
                    pieces = []
                    kmap = {}
                    for gi in range(N_CHUNK // group):
                        lt = lat_pool.tile([P, group * tpb], lat_dt,
                                           name="lt", tag="lt")
                        nc.sync.dma_start(
                            lt[:], latBs[bi][:, gi * group:(gi + 1) * group, :])
                        pieces.append(lt)
                        for i in range(group):
                            kmap[gi * group + i] = (gi, i)
                    do_i3 = drain == "i3all" or (
                        drain.startswith("i3last") and bi == len(blocks) - 1)
                    if do_i3:
                        for j in range(bsup):
                            s = s_base + j
                            nc.tensor.matmul(
                                pss[s][:], i3_t[:],
                                nz16[:, s * SUPER:(s + 1) * SUPER],
                                start=True, stop=False)
                    for k in range(N_CHUNK):
                        gi, i = kmap[k]
                        for j in range(bsup):
                            s = s_base + j
                            nc.tensor.matmul(
                                pss[s][:], wt_t[:, k * 3:(k + 1) * 3],
                                pieces[gi][:, i * tpb + j * SUPER:
                                           i * tpb + (j + 1) * SUPER],
                                start=(k == 0 and not do_i3),
                                stop=(k == N_CHUNK - 1),
                            )
                    for j in range(bsup):
                        s = s_base + j
                        eng = nc.sync if j % 2 == 0 else nc.scalar
                        osb = osb_pool.tile([3, SUPER], F32,
                                            name="osb", tag="osb")
                        if do_i3:
                            if j % 2 == 0 or drain == "i3lastv":
                                nc.vector.tensor_copy(osb[:], pss[s][:])
                            else:
                                nc.scalar.copy(osb[:], pss[s][:])
                        else:
                            nc.vector.tensor_add(
                                osb[:], pss[s][:],
                                nz_t[:, s * SUPER:(s + 1) * SUPER])
                        eng.dma_start(
                            out[:, s * SUPER:(s + 1) * SUPER], osb[:])
                    s_base += bsup

    nc.compile()
    _NC_CACHE[key] = nc
    return nc


def _coeff(T: int) -> float:
    a = 1.0
    for t in range(T):
        a *= (t + 1) / T
    return a


PIPELINE = "v11_fp8"  # "v7" | "v8_*" | "v9_*" | "v10_*" | "v11_*" | "v12_*"
_V10_GROUP = 2
_V10_BUFS = 10
_WARMUP_MMS = 6
_FINE_LEAD = False
_PAIR0_GROUP = 2
_V11_GROUP = 2
_V11_BUFS = 10
_V11_WARMUPS = 8
_V11_DRAIN = "add"    # "add" | "i3last" | "i3all"
_V11_STORE = "gpend"  # "alt" | "scalar" | "gp" | "gpend"
_V11_ISSUE = "sync"   # "sync" | "split" | "gp0"
_V11_DUMMY = 0
_V11_WTENG = "scalar"  # "scalar" | "sync" | "gp"
_V12_BS = 4
_V12_GROUP = 2
_V12_BUFS = 6
_V12_WARMUPS = 8
_V12_DRAIN = "i3last"
_V12_DUMMY = 0
_V12_LEAD = 0
_V12_WTINY = 0
_V12_WTENG = "scalar"
_V12_ISSUE = "sync"
_V12_MONO = 0
_V12_DRK = 0
_V13_PLAN = "1:2,1:2,2:2,4:2"
_V13_BUFS = 8
_V13_WARMUPS = 3
_V13_DRAIN = "i3last"
_V13_WTENG = "scalar"


def kernel(latent, W, b, noise, diffusion_steps, _trace=False, _pipeline=None):
    import ml_dtypes
    import os
    global _V11_GROUP, _V11_BUFS, _V11_WARMUPS, _V11_DRAIN, _V11_STORE
    global _V11_ISSUE, _V11_DUMMY
    _V11_GROUP = int(os.environ.get("V11_GROUP", _V11_GROUP))
    _V11_BUFS = int(os.environ.get("V11_BUFS", _V11_BUFS))
    _V11_WARMUPS = int(os.environ.get("V11_WARMUPS", _V11_WARMUPS))
    _V11_DRAIN = os.environ.get("V11_DRAIN", _V11_DRAIN)
    _V11_STORE = os.environ.get("V11_STORE", _V11_STORE)
    _V11_ISSUE = os.environ.get("V11_ISSUE", _V11_ISSUE)
    _V11_DUMMY = int(os.environ.get("V11_DUMMY", _V11_DUMMY))
    global _V11_WTENG
    _V11_WTENG = os.environ.get("V11_WTENG", _V11_WTENG)
    global _V12_BS, _V12_GROUP, _V12_BUFS, _V12_WARMUPS, _V12_DRAIN
    global _V12_DUMMY, _V12_LEAD
    _V12_BS = int(os.environ.get("V12_BS", _V12_BS))
    _V12_GROUP = int(os.environ.get("V12_GROUP", _V12_GROUP))
    _V12_BUFS = int(os.environ.get("V12_BUFS", _V12_BUFS))
    _V12_WARMUPS = int(os.environ.get("V12_WARMUPS", _V12_WARMUPS))
    _V12_DRAIN = os.environ.get("V12_DRAIN", _V12_DRAIN)
    _V12_DUMMY = int(os.environ.get("V12_DUMMY", _V12_DUMMY))
    _V12_LEAD = int(os.environ.get("V12_LEAD", _V12_LEAD))
    global _V12_WTINY, _V12_WTENG, _V12_ISSUE, _V12_MONO
    _V12_WTINY = int(os.environ.get("V12_WTINY", _V12_WTINY))
    _V12_WTENG = os.environ.get("V12_WTENG", _V12_WTENG)
    _V12_ISSUE = os.environ.get("V12_ISSUE", _V12_ISSUE)
    _V12_MONO = int(os.environ.get("V12_MONO", _V12_MONO))
    global _V12_DRK
    _V12_DRK = int(os.environ.get("V12_DRK", _V12_DRK))
    global _V13_PLAN, _V13_BUFS, _V13_WARMUPS, _V13_DRAIN, _V13_WTENG
    _V13_PLAN = os.environ.get("V13_PLAN", _V13_PLAN)
    _V13_BUFS = int(os.environ.get("V13_BUFS", _V13_BUFS))
    _V13_WARMUPS = int(os.environ.get("V13_WARMUPS", _V13_WARMUPS))
    _V13_DRAIN = os.environ.get("V13_DRAIN", _V13_DRAIN)
    _V13_WTENG = os.environ.get("V13_WTENG", _V13_WTENG)
    T = int(diffusion_steps)
    A = _coeff(T)
    pipeline = _pipeline or PIPELINE
    fp8 = pipeline.endswith("fp8")
    v9 = pipeline.startswith("v9")

    lat_flat = np.ascontiguousarray(latent.reshape(TOK, D), dtype=np.float32)
    if fp8:
        latT_h = lat_flat.astype(ml_dtypes.float8_e3m4).T  # [D, TOK] view
    else:
        latT_h = lat_flat.astype(np.float16).T
    wt_eff = np.ascontiguousarray(W.T).astype(np.float32) * np.float32(1.0 - A)
    # prepack [2048, 3] -> [128, 16*3]: chunk k (rows 128k..128k+128) at cols 3k..3k+3
    wt_packed = np.ascontiguousarray(
        wt_eff.reshape(N_CHUNK, P, 3).transpose(1, 0, 2).reshape(P, 3 * N_CHUNK)
    ).astype(np.float16)
    nz_eff = (np.float32(A) * noise.reshape(TOK, 3)
              + np.float32(1.0 - A) * b[None, :].astype(np.float32))
    nz_eff_t = np.ascontiguousarray(nz_eff.T.astype(np.float32))  # [3, TOK]

    lat_dt = mybir.dt.float8e3 if fp8 else F16
    v10 = pipeline.startswith("v10")
    v11 = pipeline.startswith("v11")
    v12 = pipeline.startswith("v12")
    v13 = pipeline.startswith("v13")
    if pipeline == "v7":
        nc = _build_nc_v7()
    elif v13:
        nc = _build_nc_v13(lat_dt=lat_dt, plan=_V13_PLAN, bufs=_V13_BUFS,
                           warmups=_V13_WARMUPS, drain=_V13_DRAIN,
                           wt_eng=_V13_WTENG)
    elif v12:
        nc = _build_nc_v12(lat_dt=lat_dt, bs=_V12_BS, group=_V12_GROUP,
                           bufs=_V12_BUFS, warmups=_V12_WARMUPS,
                           drain=_V12_DRAIN, dummy=_V12_DUMMY, lead=_V12_LEAD,
                           wtiny=_V12_WTINY, wt_eng=_V12_WTENG,
                           issue=_V12_ISSUE, mono=_V12_MONO, drk=_V12_DRK)
    elif v11:
        nc = _build_nc_v11(lat_dt=lat_dt, group=_V11_GROUP, bufs=_V11_BUFS,
                           warmups=_V11_WARMUPS, drain=_V11_DRAIN,
                           store_eng=_V11_STORE, issue=_V11_ISSUE,
                           dummy=_V11_DUMMY, wt_eng=_V11_WTENG)
    elif v10:
        nc = _build_nc_v10(lat_dt=lat_dt, group=_V10_GROUP, bufs=_V10_BUFS)
    elif v9:
        nc = _build_nc_v9(lat_dt=lat_dt)
    else:
        nc = _build_nc_v8(lat_dt=lat_dt)
    if v10:
        # [D, TOK] -> [128, 16, TOK]: partition-major chunk layout
        lat_p = np.ascontiguousarray(
            latT_h.reshape(N_CHUNK, P, TOK).transpose(1, 0, 2))
    if v11 or v12 or v13:
        # [TOK, D] fp8 -> per-core [block*128, 16, blocktok]: piece
        # (block, k-range) is contiguous per partition line in DRAM
        lat8 = latT_h.T  # the untransposed contiguous [TOK, D] downcast
        if not v13:
            blk_tok = PIECE if v11 else _V12_BS * SUPER
            n_blk = TPC // blk_tok
        drain_mode = (_V11_DRAIN if v11 else
                      _V12_DRAIN if v12 else _V13_DRAIN)
        if v13:
            v13_blocks = [int(p.split(":")[0]) for p in _V13_PLAN.split(",")]
    in_maps = []
    for c in range(N_CORES):
        if v13:
            shard = lat8[c * TPC:(c + 1) * TPC]          # [4096, 2048]
            m = {"wt": wt_packed,
                 "nzt": np.ascontiguousarray(nz_eff_t[:, c * TPC:(c + 1) * TPC])}
            t0 = 0
            for bi, bsup in enumerate(v13_blocks):
                tpb = bsup * SUPER
                sub = shard[t0:t0 + tpb]                  # [tpb, 2048]
                m[f"latT{bi}"] = np.ascontiguousarray(
                    sub.reshape(tpb, N_CHUNK, P).transpose(2, 1, 0))
                t0 += tpb
            if drain_mode.startswith("i3"):
                m["nzt16"] = m["nzt"].astype(np.float16)
                m["i3"] = np.eye(3, dtype=np.float16)
            in_maps.append(m)
            continue
        if v11 or v12:
            drk = _V12_DRK if v12 else 0
            nplain = N_CHUNK - 2 * drk
            shard = lat8[c * TPC:(c + 1) * TPC]          # [4096, 2048]
            lat_c = np.ascontiguousarray(
                shard[:, :nplain * P]
                .reshape(n_blk, blk_tok, nplain, P)
                .transpose(0, 3, 2, 1)
                .reshape(n_blk * P, nplain, blk_tok))
            m = {"latT": lat_c, "wt": wt_packed,
                 "nzt": np.ascontiguousarray(nz_eff_t[:, c * TPC:(c + 1) * TPC])}
            if drk:
                # DoubleRow chunks: e4m3 straight from the f32 source
                shard4 = lat_flat[c * TPC:(c + 1) * TPC, nplain * P:].astype(
                    ml_dtypes.float8_e4m3)
                m["latD"] = np.ascontiguousarray(
                    shard4.reshape(n_blk, blk_tok, 2 * drk, P)
                    .transpose(0, 3, 2, 1)
                    .reshape(n_blk * P, 2 * drk, blk_tok))
                wt4_h = np.zeros((P, 2 * drk, 16), dtype=ml_dtypes.float8_e4m3)
                wt4_h[:, :, 0:3] = (
                    wt_eff[nplain * P:]
                    .reshape(2 * drk, P, 3).transpose(1, 0, 2)
                    .astype(ml_dtypes.float8_e4m3))
                m["wt4"] = wt4_h
            if drain_mode.startswith("i3"):
                m["nzt16"] = m["nzt"].astype(np.float16)
                m["i3"] = np.eye(3, dtype=np.float16)
            in_maps.append(m)
            continue
        in_maps.append({
            "latT": (np.ascontiguousarray(lat_p[:, :, c * TPC:(c + 1) * TPC])
                     if v10 else
                     np.ascontiguousarray(latT_h[:, c * TPC:(c + 1) * TPC])),
            "wt": wt_packed,
            "nzt": np.ascontiguousarray(nz_eff_t[:, c * TPC:(c + 1) * TPC]),
        })
    res = run_bass_kernel_spmd(nc, in_maps, core_ids=list(range(N_CORES)),
                               trace=_trace)
    out = np.empty((TOK, 3), dtype=np.float32)
    for c in range(N_CORES):
        out[c * TPC:(c + 1) * TPC] = res.results[c]["out"].T
    if _trace:
        kernel._last_results = res
    return out.reshape(B, S, 3)



# revision 86
# speedup vs baseline: 1.1819x; 1.0082x over previous
"""Trainium2 Bass kernel for DiffusionCoordinateInitializer.

Reference computation:
    coords = einsum("bsd,cd->bsc", latent, W) + b          # [B, S, 3]
    x = noise; for t in reversed(range(T)): x = a*x + (1-a)*coords, a=(t+1)/T
which collapses (affine fixed-point iteration) to
    x = A*noise + (1-A)*(coords + b),  A = prod_{t=1..T} t/T = T!/T^T

Strategy (pure data-parallel over 8 cores, token-sharded; v11 pipeline):
  - Host folds (1-A) into W^T and A*noise + (1-A)*b into a bias tensor, so
    the device computes out^T[3, tok] = (W_eff @ latent^T) + bias^T.
  - Host pre-transposes + downcasts latent to fp8 e3m4 in a PAIR-BLOCKED
    partition-major layout [pair*128, 16 chunks, 1024 tok]: each DMA piece
    [128, 2, 1024] reads a contiguous 2 KB run per partition (2 KB DMA
    descriptors instead of the 1 KB the old [128,16,TOK] layout forced),
    1/4 the HBM traffic of fp32 (8.4 MB/core, ~24 us DMA floor). e3m4
    moving x fp16 stationary matmuls keep max-rel-err at 1.3e-2 (gate 2e-2).
  - Token-pair-major schedule: each 1024-token pair streams 8 two-chunk
    [128, 2048] pieces and runs 32 skinny matmuls (W chunk stationary
    [128, 3], moving [128, 512], 1 cyc/row) accumulating into 2 PSUM
    banks; its bias-add (DVE) + [3, 512] store drain while the next pair
    streams, so only the last pair's drain is exposed.
  - 8 warmup matmuls off a memset scratch tile ramp the PE clock from
    ~7.4 us (engines live) to the first real matmul ~11 us, so real work
    runs at full 2.4 GHz immediately; a gap here would reset the ramp
    (measured: a 1.1 us gap costs ~5 us, so warmups err on the long side).
  - Output stores ride the GpSimd SWDGE ("gpend"), keeping the Sync
    sequencer 100% dedicated to latent DIRECT2Ds mid-stream (-1.5 us,
    zero matmul gaps); the last pair's two stores switch to the by-then
    idle Sync+Scalar HWDGEs so they issue in parallel instead of
    serializing on the single SWDGE queue (-0.9 us of exposed tail).
Steady state is a razor-thin race: the PE consumes fp8 at ~307 GB/s vs
~330-370 GB/s DMA delivery, so the ~1 MB lookahead banked during warmup
(bufs=10) is the shock absorber that keeps the PE gap-free.
Measured: 45.1-46.4 us/core on trn2 (quiet device; congested windows add
2-8 us). exec = ~6.5 us fixed NEFF preamble + ~3.6 us warmup/data-wait +
~28.5 us gap-free PE stream + ~2.2 us drain/stores + ~2.9 us teardown.
Rejected by experiment: i3-matmul noise folding (2 extra PE matmuls cost
more than the tail saves), issue-split across Sync+Scalar DGEs
(out-of-order delivery vs in-order consumption), chunk-major 2 MB pieces
(PE starvation -> p-state collapse), ascending block plans, 64-row warmup
granules, SWDGE piece-0, wt via sync/gpsimd, and partial e4m3 DoubleRow
(only -0.5 us for rel-err 1.96e-2 -- too close to the 2e-2 gate).
"""

import numpy as np
from contextlib import ExitStack

import concourse.bass as bass  # noqa: F401
import concourse.tile as tile
from concourse import bacc, mybir
from concourse.bass_utils import run_bass_kernel_spmd

N_CORES = 8
B, S, D = 4, 8192, 2048
TOK = B * S                      # 32768
TPC = TOK // N_CORES             # 4096 tokens per core
P = 128
SUPER = 512                      # tokens per PSUM bank (max psum free f32)
N_SUPER = TPC // SUPER           # 8
N_CHUNK = D // P                 # 16
F32 = mybir.dt.float32
F16 = mybir.dt.float16
F8E3 = mybir.dt.float8e3

_NC_CACHE = {}


def _build_nc_v7(lat_dt=F16, repeat=1):
    """Pre-transposed stream: latT [D, TPC] (fp16 or fp8e3m4) in DRAM,
    chunk-major accumulating matmuls into 8 PSUM banks, no transposes."""
    key = ("v7", lat_dt, repeat)
    if key in _NC_CACHE:
        return _NC_CACHE[key]

    nc = bacc.Bacc("TRN2", target_bir_lowering=False, debug=False,
                   enable_asserts=False, num_devices=N_CORES)
    latT = nc.dram_tensor("latT", [D, TPC], lat_dt, kind="ExternalInput").ap()
    # host prepacks W_eff^T chunks as [128, 16*3]: wt[p, 3k+c] = W_eff[c, 128k+p]
    wt = nc.dram_tensor("wt", [P, 3 * N_CHUNK], F16, kind="ExternalInput").ap()
    nzt = nc.dram_tensor("nzt", [3, TPC], F32, kind="ExternalInput").ap()
    out = nc.dram_tensor("out", [3, TPC], F32, kind="ExternalOutput").ap()

    with tile.TileContext(nc) as tc:
        with ExitStack() as ctx:
            const = ctx.enter_context(tc.tile_pool(name="const", bufs=1))
            lat_pool = ctx.enter_context(tc.tile_pool(name="lat", bufs=4))
            ps_pool = ctx.enter_context(tc.tile_pool(name="ps", bufs=1, space="PSUM"))
            osb_pool = ctx.enter_context(tc.tile_pool(name="osb", bufs=2))

            wt_t = const.tile([P, 3 * N_CHUNK], F16)
            nc.sync.dma_start(wt_t[:], wt[:])
            nz_t = const.tile([3, TPC], F32)
            nc.sync.dma_start(nz_t[:], nzt[:])

            for _ in range(repeat):
                pss = [ps_pool.tile([3, SUPER], F32, name=f"ps{s}", tag=f"ps{s}")
                       for s in range(N_SUPER)]
                for k in range(N_CHUNK):
                    lt = lat_pool.tile([P, TPC], F16, name="lt", tag="lt")
                    nc.sync.dma_start(lt[:], latT[k * P:(k + 1) * P, :])
                    for s in range(N_SUPER):
                        nc.tensor.matmul(
                            pss[s][:], wt_t[:, k * 3:(k + 1) * 3],
                            lt[:, s * SUPER:(s + 1) * SUPER],
                            start=(k == 0), stop=(k == N_CHUNK - 1),
                        )
                osb = osb_pool.tile([3, TPC], F32, name="osb", tag="osb")
                for s in range(N_SUPER):
                    nc.vector.tensor_add(osb[:, s * SUPER:(s + 1) * SUPER],
                                         pss[s][:], nz_t[:, s * SUPER:(s + 1) * SUPER])
                nc.sync.dma_start(out[:], osb[:])

    nc.compile()
    _NC_CACHE[key] = nc
    return nc


PIECE = 1024                     # tokens per DMA piece (2 KB/part fp16)
N_PIECE = TPC // PIECE           # 4 pieces per chunk


def _build_nc_v8(lat_dt=F16, wt_dt=F16, repeat=1):
    """Piece-granular stream + interleaved drain.

    Same math as v7 but: each 128-row d-chunk is loaded as 4 [128, 1024]
    pieces so the first matmul starts ~8 us earlier; after the last chunk,
    each super's bias-add runs on alternating Vector/Scalar engines right
    behind its stop-matmul, and its [3, 512] store issues immediately --
    the drain hides under the PE tail instead of serializing after it.
    """
    key = ("v8", lat_dt, wt_dt, repeat)
    if key in _NC_CACHE:
        return _NC_CACHE[key]

    nc = bacc.Bacc("TRN2", target_bir_lowering=False, debug=False,
                   enable_asserts=False, num_devices=N_CORES)
    latT = nc.dram_tensor("latT", [D, TPC], lat_dt, kind="ExternalInput").ap()
    wt = nc.dram_tensor("wt", [P, 3 * N_CHUNK], wt_dt, kind="ExternalInput").ap()
    nzt = nc.dram_tensor("nzt", [3, TPC], F32, kind="ExternalInput").ap()
    out = nc.dram_tensor("out", [3, TPC], F32, kind="ExternalOutput").ap()

    SPP = PIECE // SUPER  # supers per piece (2)

    with tile.TileContext(nc) as tc:
        with ExitStack() as ctx:
            const = ctx.enter_context(tc.tile_pool(name="const", bufs=1))
            lat_pool = ctx.enter_context(tc.tile_pool(name="lat", bufs=16))
            ps_pool = ctx.enter_context(tc.tile_pool(name="ps", bufs=1, space="PSUM"))
            osb_pool = ctx.enter_context(tc.tile_pool(name="osb", bufs=8))

            # consts via engine sequencers: the Sync sequencer spends the
            # first ~9 us on queue init, and a DIRECT2D issued there would
            # gate the first matmul on the weights until ~11 us.
            wt_t = const.tile([P, 3 * N_CHUNK], wt_dt)
            nc.scalar.dma_start(wt_t[:], wt[:])
            nz_t = const.tile([3, TPC], F32)
            nc.gpsimd.dma_start(nz_t[:], nzt[:])

            for _ in range(repeat):
                pss = [ps_pool.tile([3, SUPER], F32, name=f"ps{s}", tag=f"ps{s}")
                       for s in range(N_SUPER)]
                for k in range(N_CHUNK):
                    # chunk 0 in 512-token pieces so the first matmul's
                    # dependency lands ~2 us after DMA start; 1024 after
                    w = SUPER if k == 0 else PIECE
                    spp = w // SUPER
                    pieces = []
                    for p in range(TPC // w):
                        lt = lat_pool.tile([P, w], lat_dt, name="lt", tag="lt")
                        nc.sync.dma_start(
                            lt[:], latT[k * P:(k + 1) * P, p * w:(p + 1) * w])
                        pieces.append(lt)
                    for s in range(N_SUPER):
                        nc.tensor.matmul(
                            pss[s][:], wt_t[:, k * 3:(k + 1) * 3],
                            pieces[s // spp][:, (s % spp) * SUPER:
                                             (s % spp + 1) * SUPER],
                            start=(k == 0), stop=(k == N_CHUNK - 1),
                        )
                        if k == N_CHUNK - 1:
                            osb = osb_pool.tile([3, SUPER], F32,
                                                name="osb", tag="osb")
                            nc.vector.tensor_add(osb[:], pss[s][:],
                                                 nz_t[:, s * SUPER:(s + 1) * SUPER])
                            nc.scalar.dma_start(
                                out[:, s * SUPER:(s + 1) * SUPER], osb[:])

    nc.compile()
    _NC_CACHE[key] = nc
    return nc


def _build_nc_v10(lat_dt=F16, wt_dt=F16, group=2, bufs=6, repeat=1):
    """Pair-major with multi-chunk DMA pieces.

    latT3 [128, 16, TPC] host layout (partition-major) lets one DMA carry
    `group` chunks for a 1024-token pair: [128, group, 1024] -> SBUF
    [128, group*1024]. Fewer, bigger transfers = fewer PE semaphore waits
    (the ~0.2 us/piece stall tax v9 measured with 64 pieces).
    """
    key = ("v10", lat_dt, wt_dt, group, bufs, repeat, _WARMUP_MMS, _FINE_LEAD, _PAIR0_GROUP)
    if key in _NC_CACHE:
        return _NC_CACHE[key]

    nc = bacc.Bacc("TRN2", target_bir_lowering=False, debug=False,
                   enable_asserts=False, num_devices=N_CORES)
    latT3 = nc.dram_tensor("latT", [P, N_CHUNK, TPC], lat_dt,
                           kind="ExternalInput").ap()
    wt = nc.dram_tensor("wt", [P, 3 * N_CHUNK], wt_dt, kind="ExternalInput").ap()
    nzt = nc.dram_tensor("nzt", [3, TPC], F32, kind="ExternalInput").ap()
    out = nc.dram_tensor("out", [3, TPC], F32, kind="ExternalOutput").ap()

    NG = N_CHUNK // group

    with tile.TileContext(nc) as tc:
        with ExitStack() as ctx:
            const = ctx.enter_context(tc.tile_pool(name="const", bufs=1))
            lat_pool = ctx.enter_context(tc.tile_pool(name="lat", bufs=bufs))
            ps_pool = ctx.enter_context(tc.tile_pool(name="ps", bufs=1, space="PSUM"))
            osb_pool = ctx.enter_context(tc.tile_pool(name="osb", bufs=4))

            wt_t = const.tile([P, 3 * N_CHUNK], wt_dt)
            nc.scalar.dma_start(wt_t[:], wt[:])
            nz_t = const.tile([3, TPC], F32)
            nc.gpsimd.dma_start(nz_t[:], nzt[:])

            # p-state warmup: a few throwaway matmuls on a memset scratch
            # tile while the first latent pieces are still in flight, so the
            # PE clock ramps before real work starts. Results land in bank 7,
            # which that super's real group resets with start=True.
            warm = const.tile([P, SUPER], wt_dt)
            nc.vector.memset(warm[:], 1.0)

            for _ in range(repeat):
                pss = [ps_pool.tile([3, SUPER], F32, name=f"ps{s}", tag=f"ps{s}")
                       for s in range(N_SUPER)]
                # warmups wait on wt_t (~8.9 us) and end right as the first
                # latent piece lands (~11.4), carrying the clock ramp into
                # real work with no idle gap (a gap would reset the ramp;
                # starting them earlier off a self-operand measured worse)
                for _ in range(_WARMUP_MMS):
                    nc.tensor.matmul(pss[7][:], wt_t[:, 0:3], warm[:],
                                     start=True, stop=True)
                for pr in range(N_SUPER // 2):
                    # uniform piece sizes: every non-uniform variant
                    # (fine lead pieces, single-chunk pair 0) measured worse --
                    # concurrent DMAs complete fair-share, so mixed sizes delay
                    # the bulk pieces and idle gaps reset the PE clock ramp
                    grp = _PAIR0_GROUP if pr == 0 else group
                    if pr == 0 and _FINE_LEAD:
                        sizes = [1, 1] + [group] * ((N_CHUNK - 2) // group)
                    else:
                        sizes = [grp] * (N_CHUNK // grp)
                    kmap = {}
                    k0 = 0
                    for gi, sz in enumerate(sizes):
                        for i in range(sz):
                            kmap[k0 + i] = (gi, i)
                        k0 += sz
                    pieces = []
                    off = 0
                    for gi, sz in enumerate(sizes):
                        lt = lat_pool.tile([P, sz * PIECE], lat_dt,
                                           name="lt", tag="lt")
                        nc.sync.dma_start(
                            lt[:], latT3[:, off:off + sz,
                                         pr * PIECE:(pr + 1) * PIECE])
                        pieces.append(lt)
                        off += sz
                    for k in range(N_CHUNK):
                        g, i = kmap[k]
                        for j in range(2):
                            s = 2 * pr + j
                            nc.tensor.matmul(
                                pss[s][:], wt_t[:, k * 3:(k + 1) * 3],
                                pieces[g][:, i * PIECE + j * SUPER:
                                         i * PIECE + (j + 1) * SUPER],
                                start=(k == 0), stop=(k == N_CHUNK - 1),
                            )
                    for j in range(2):
                        s = 2 * pr + j
                        osb = osb_pool.tile([3, SUPER], F32, name="osb", tag="osb")
                        nc.vector.tensor_add(osb[:], pss[s][:],
                                             nz_t[:, s * SUPER:(s + 1) * SUPER])
                        eng = nc.sync if j == 0 else nc.scalar
                        eng.dma_start(
                            out[:, s * SUPER:(s + 1) * SUPER], osb[:])

    nc.compile()
    _NC_CACHE[key] = nc
    return nc


def _build_nc_v9(lat_dt=F16, wt_dt=F16, repeat=1):
    """v8 + bias-add folded into the PE and stores straight from PSUM.

    The noise/bias term enters each super's accumulation group as one extra
    matmul: stationary = I3 [3, 3], moving = nz16 [3, 512] fp16, so
    psum += I3^T @ nz = nz elementwise. No Vector/Scalar engine work at
    all; each super's [3, 512] result DMAs from PSUM as soon as its group
    stops, hiding the whole drain under the PE tail.
    """
    key = ("v9", lat_dt, wt_dt, repeat)
    if key in _NC_CACHE:
        return _NC_CACHE[key]

    nc = bacc.Bacc("TRN2", target_bir_lowering=False, debug=False,
                   enable_asserts=False, num_devices=N_CORES)
    latT = nc.dram_tensor("latT", [D, TPC], lat_dt, kind="ExternalInput").ap()
    wt = nc.dram_tensor("wt", [P, 3 * N_CHUNK], wt_dt, kind="ExternalInput").ap()
    nzt = nc.dram_tensor("nzt", [3, TPC], F32, kind="ExternalInput").ap()
    out = nc.dram_tensor("out", [3, TPC], F32, kind="ExternalOutput").ap()

    with tile.TileContext(nc) as tc:
        with ExitStack() as ctx:
            const = ctx.enter_context(tc.tile_pool(name="const", bufs=1))
            lat_pool = ctx.enter_context(tc.tile_pool(name="lat", bufs=32))
            ps_pool = ctx.enter_context(tc.tile_pool(name="ps", bufs=1, space="PSUM"))
            osb_pool = ctx.enter_context(tc.tile_pool(name="osb", bufs=4))

            wt_t = const.tile([P, 3 * N_CHUNK], wt_dt)
            nc.scalar.dma_start(wt_t[:], wt[:])
            nz_t = const.tile([3, TPC], F32)
            nc.gpsimd.dma_start(nz_t[:], nzt[:])

            for _ in range(repeat):
                pss = [ps_pool.tile([3, SUPER], F32, name=f"ps{s}", tag=f"ps{s}")
                       for s in range(N_SUPER)]
                # token-pair-major: each 1024-token pair streams all 16
                # chunks, closes its two accumulation groups, and drains
                # while the next pair streams -- no end-of-kernel drain.
                for pr in range(N_SUPER // 2):
                    pieces = []
                    for k in range(N_CHUNK):
                        lt = lat_pool.tile([P, PIECE], lat_dt, name="lt", tag="lt")
                        nc.sync.dma_start(
                            lt[:], latT[k * P:(k + 1) * P,
                                        pr * PIECE:(pr + 1) * PIECE])
                        pieces.append(lt)
                    for k in range(N_CHUNK):
                        for j in range(2):
                            s = 2 * pr + j
                            nc.tensor.matmul(
                                pss[s][:], wt_t[:, k * 3:(k + 1) * 3],
                                pieces[k][:, j * SUPER:(j + 1) * SUPER],
                                start=(k == 0), stop=(k == N_CHUNK - 1),
                            )
                    for j in range(2):
                        s = 2 * pr + j
                        osb = osb_pool.tile([3, SUPER], F32, name="osb", tag="osb")
                        nc.vector.tensor_add(osb[:], pss[s][:],
                                             nz_t[:, s * SUPER:(s + 1) * SUPER])
                        eng = nc.sync if j == 0 else nc.scalar
                        eng.dma_start(
                            out[:, s * SUPER:(s + 1) * SUPER], osb[:])

    nc.compile()
    _NC_CACHE[key] = nc
    return nc


def _build_nc_v11(lat_dt=F8E3, wt_dt=F16, group=2, bufs=10, warmups=8,
                  drain="add", store_eng="alt", issue="sync", dummy=0,
                  wt_eng="scalar", repeat=1):
    """v10 + pair-contiguous DRAM layout + decoupled warmups.

    latP [N_PAIR*128, 16, 1024] host layout: piece (pr, g0:g0+sz) reads
    latP[pr*128:(pr+1)*128, g0:g0+sz, :] whose per-partition run is
    sz KB *contiguous* in DRAM -> sz-KB DMA descriptors instead of the
    1 KB forced by the old [128, 16, TPC] layout (4x fewer descriptors at
    group=4: less DGE issue time on Sync, less per-descriptor queue tax).

    Warmup matmuls take BOTH operands from the memset scratch tile, so
    they start as soon as the Tensor sequencer is live (~6.2 us) instead
    of waiting for the weight DMA (~8.9 us): the PE clock ramp completes
    before the first real matmul, shaving the pstate tax off real work.
    """
    key = ("v11", lat_dt, wt_dt, group, bufs, warmups, drain, store_eng,
           issue, dummy, wt_eng, repeat)
    if key in _NC_CACHE:
        return _NC_CACHE[key]

    nc = bacc.Bacc("TRN2", target_bir_lowering=False, debug=False,
                   enable_asserts=False, num_devices=N_CORES)
    N_PAIR = N_SUPER // 2
    latP = nc.dram_tensor("latT", [N_PAIR * P, N_CHUNK, PIECE], lat_dt,
                          kind="ExternalInput").ap()
    wt = nc.dram_tensor("wt", [P, 3 * N_CHUNK], wt_dt, kind="ExternalInput").ap()
    nzt = nc.dram_tensor("nzt", [3, TPC], F32, kind="ExternalInput").ap()
    if drain.startswith("i3"):
        nzt16 = nc.dram_tensor("nzt16", [3, TPC], F16, kind="ExternalInput").ap()
        i3 = nc.dram_tensor("i3", [3, 3], F16, kind="ExternalInput").ap()
    out = nc.dram_tensor("out", [3, TPC], F32, kind="ExternalOutput").ap()

    NG = N_CHUNK // group

    with tile.TileContext(nc) as tc:
        with ExitStack() as ctx:
            const = ctx.enter_context(tc.tile_pool(name="const", bufs=1))
            lat_pool = ctx.enter_context(tc.tile_pool(name="lat", bufs=bufs))
            ps_pool = ctx.enter_context(tc.tile_pool(name="ps", bufs=1, space="PSUM"))
            osb_pool = ctx.enter_context(tc.tile_pool(name="osb", bufs=4))

            # memset FIRST so the warmup matmuls (gated only on it) start
            # the moment the engines come out of the init barrier
            warm = const.tile([P, SUPER], wt_dt)
            nc.vector.memset(warm[:], 0.001)

            if dummy:
                # 1-descriptor lead DMAs absorb the first-use queue-start
                # latency so piece 0's descriptors find live queues
                dmy = const.tile([1, 4], wt_dt)
                nc.sync.dma_start(dmy[:], wt[0:1, 0:4])
                dmy2 = const.tile([1, 4], wt_dt)
                nc.scalar.dma_start(dmy2[:], wt[0:1, 0:4])

            wt_t = const.tile([P, 3 * N_CHUNK], wt_dt)
            {"scalar": nc.scalar, "sync": nc.sync,
             "gp": nc.gpsimd}[wt_eng].dma_start(wt_t[:], wt[:])
            nz_t = const.tile([3, TPC], F32)
            nc.gpsimd.dma_start(nz_t[:], nzt[:])
            if drain.startswith("i3"):
                # identity [3,3] (host-supplied): psum += I3^T @ nz16
                nz16 = const.tile([3, TPC], F16)
                nc.gpsimd.dma_start(nz16[:], nzt16[:])
                i3_t = const.tile([3, 3], F16)
                nc.scalar.dma_start(i3_t[:], i3[:])

            for _ in range(repeat):
                pss = [ps_pool.tile([3, SUPER], F32, name=f"ps{s}", tag=f"ps{s}")
                       for s in range(N_SUPER)]
                # warmups gated only on the memset: start ~6.2 us, ramp the
                # PE clock while wt + piece 0 are in flight
                for _ in range(warmups):
                    nc.tensor.matmul(pss[7][:], warm[:, 0:3], warm[:],
                                     start=True, stop=True)
                n_piece = 0
                for pr in range(N_PAIR):
                    pieces = []
                    for gi in range(NG):
                        lt = lat_pool.tile([P, group * PIECE], lat_dt,
                                           name="lt", tag="lt")
                        if issue == "split":
                            eng = nc.sync if n_piece % 2 == 0 else nc.scalar
                        elif issue == "gp0" and n_piece == 0:
                            # piece 0 via the GpSimd SWDGE path: its
                            # sequencer can issue ~1.3 us before Sync's
                            # first DIRECT2D, pulling the first real
                            # matmul's dependency earlier
                            eng = nc.gpsimd
                        else:
                            eng = nc.sync
                        eng.dma_start(
                            lt[:], latP[pr * P:(pr + 1) * P,
                                        gi * group:(gi + 1) * group, :])
                        pieces.append(lt)
                        n_piece += 1
                    do_i3 = drain == "i3all" or (
                        drain == "i3last" and pr == N_PAIR - 1)
                    if do_i3:
                        for s in (2 * pr, 2 * pr + 1):
                            nc.tensor.matmul(
                                pss[s][:], i3_t[:],
                                nz16[:, s * SUPER:(s + 1) * SUPER],
                                start=True, stop=False)
                    for k in range(N_CHUNK):
                        g, i = divmod(k, group)
                        for j in range(2):
                            s = 2 * pr + j
                            nc.tensor.matmul(
                                pss[s][:], wt_t[:, k * 3:(k + 1) * 3],
                                pieces[g][:, i * PIECE + j * SUPER:
                                         i * PIECE + (j + 1) * SUPER],
                                start=(k == 0 and not do_i3),
                                stop=(k == N_CHUNK - 1),
                            )
                    for j in range(2):
                        s = 2 * pr + j
                        if store_eng == "alt":
                            eng = nc.sync if j == 0 else nc.scalar
                        elif store_eng == "gp":
                            eng = nc.gpsimd
                        elif store_eng == "gpend":
                            # gp SWDGE keeps Sync free mid-stream; the last
                            # pair's two stores go out on the now-idle
                            # Sync+Scalar HWDGEs in parallel (the single
                            # SWDGE queue would serialize them at the tail)
                            if pr == N_SUPER // 2 - 1:
                                eng = nc.sync if j == 0 else nc.scalar
                            else:
                                eng = nc.gpsimd
                        else:
                            eng = nc.scalar
                        osb = osb_pool.tile([3, SUPER], F32,
                                            name="osb", tag="osb")
                        if do_i3:
                            # noise already in PSUM via I3 matmul: pure
                            # copies, split across Vector + Scalar so the
                            # last pair's two supers drain in parallel
                            if j == 0:
                                nc.vector.tensor_copy(osb[:], pss[s][:])
                            else:
                                nc.scalar.copy(osb[:], pss[s][:])
                        else:
                            nc.vector.tensor_add(
                                osb[:], pss[s][:],
                                nz_t[:, s * SUPER:(s + 1) * SUPER])
                        eng.dma_start(
                            out[:, s * SUPER:(s + 1) * SUPER], osb[:])

    nc.compile()
    _NC_CACHE[key] = nc
    return nc


F8E4 = mybir.dt.float8e4


def _build_nc_v12(lat_dt=F8E3, wt_dt=F16, bs=4, group=2, bufs=6, warmups=8,
                  drain="i3last", dummy=0, lead=0, wtiny=0, wt_eng="scalar",
                  issue="sync", mono=0, drk=0, repeat=1):
    """Block-major: bs supers (bs*512 tokens) per block, chunk-group pieces.

    Each DMA piece is [128, group, bs*512] with a contiguous
    group*bs*512-byte run per partition: at bs=4/group=2 that is 16
    DIRECT2D issues of 512 KB (vs v11's 32+ of 256 KB) -- the Sync
    sequencer's ~0.65 us/issue serialization stops pacing the stream.
    Drains of a block's banks hide under the next block's matmuls; the
    last block's banks get the noise folded in via I3 matmuls so their
    drains are engine-parallel copies.
    lead>0: the first `lead` chunk-pieces of block 0 are single-chunk so
    the first matmul's dependency lands early.
    """
    key = ("v12", lat_dt, wt_dt, bs, group, bufs, warmups, drain, dummy,
           lead, wtiny, wt_eng, issue, mono, drk, repeat)
    if key in _NC_CACHE:
        return _NC_CACHE[key]

    nc = bacc.Bacc("TRN2", target_bir_lowering=False, debug=False,
                   enable_asserts=False, num_devices=N_CORES)
    NB = N_SUPER // bs
    TPB = bs * SUPER
    NPLAIN = N_CHUNK - 2 * drk  # chunks streamed e3m4 at 1 cyc/row
    latB = nc.dram_tensor("latT", [NB * P, NPLAIN, TPB], lat_dt,
                          kind="ExternalInput").ap()
    if drk:
        # last 2*drk chunks in e4m3 for DoubleRow (0.5 cyc/row) matmuls.
        # Weight k-tile stride must be a multiple of 16 (s3_lw dual-fp8
        # ISA restriction), so each k-tile's 3 columns sit in a 16-wide slot.
        latD = nc.dram_tensor("latD", [NB * P, 2 * drk, TPB], F8E4,
                              kind="ExternalInput").ap()
        wt4 = nc.dram_tensor("wt4", [P, 2 * drk, 16], F8E4,
                             kind="ExternalInput").ap()
    wt = nc.dram_tensor("wt", [P, 3 * N_CHUNK], wt_dt, kind="ExternalInput").ap()
    nzt = nc.dram_tensor("nzt", [3, TPC], F32, kind="ExternalInput").ap()
    if drain.startswith("i3"):
        nzt16 = nc.dram_tensor("nzt16", [3, TPC], F16, kind="ExternalInput").ap()
        i3 = nc.dram_tensor("i3", [3, 3], F16, kind="ExternalInput").ap()
    out = nc.dram_tensor("out", [3, TPC], F32, kind="ExternalOutput").ap()

    with tile.TileContext(nc) as tc:
        with ExitStack() as ctx:
            const = ctx.enter_context(tc.tile_pool(name="const", bufs=1))
            lat_pool = ctx.enter_context(tc.tile_pool(name="lat", bufs=bufs))
            ps_pool = ctx.enter_context(tc.tile_pool(name="ps", bufs=1, space="PSUM"))
            osb_pool = ctx.enter_context(tc.tile_pool(name="osb", bufs=4))

            warm = const.tile([P, SUPER], wt_dt)
            nc.vector.memset(warm[:], 0.001)

            if dummy:
                dmy = const.tile([1, 4], wt_dt)
                nc.sync.dma_start(dmy[:], wt[0:1, 0:4])

            wt_t = const.tile([P, 3 * N_CHUNK], wt_dt)
            (nc.sync if (wt_eng == "sync" or mono) else nc.scalar).dma_start(
                wt_t[:], wt[:])
            if drk:
                wt4_t = const.tile([P, 2 * drk, 16], F8E4)
                (nc.sync if mono else nc.scalar).dma_start(wt4_t[:], wt4[:])
            nz_t = const.tile([3, TPC], F32)
            (nc.sync if mono else nc.gpsimd).dma_start(nz_t[:], nzt[:])
            if drain.startswith("i3"):
                nz16 = const.tile([3, TPC], F16)
                (nc.sync if mono else nc.gpsimd).dma_start(nz16[:], nzt16[:])
                i3_t = const.tile([3, 3], F16)
                (nc.sync if mono else nc.scalar).dma_start(i3_t[:], i3[:])

            for _ in range(repeat):
                pss = [ps_pool.tile([3, SUPER], F32, name=f"ps{s}", tag=f"ps{s}")
                       for s in range(N_SUPER)]
                for _ in range(warmups):
                    nc.tensor.matmul(pss[N_SUPER - 1][:], warm[:, 0:3], warm[:],
                                     start=True, stop=True)
                # tiny tail warmups: keep the PE busy in ~27-55 ns granules
                # so a late piece 0 can't open a ramp-resetting gap, while an
                # early piece 0 only waits out the current granule
                for _ in range(wtiny):
                    nc.tensor.matmul(pss[N_SUPER - 1][:, 0:64], warm[:, 0:3],
                                     warm[:, 0:64], start=True, stop=True)
                n_piece = 0
                for b in range(NB):
                    # piece plan: optionally single-chunk leads on block 0
                    sizes = []
                    rem = NPLAIN
                    if b == 0 and lead:
                        sizes += [1] * lead
                        rem -= lead
                    sizes += [group] * (rem // group)
                    pieces = []
                    kmap = {}
                    k0 = 0
                    for gi, sz in enumerate(sizes):
                        lt = lat_pool.tile([P, sz * TPB], lat_dt,
                                           name="lt", tag="lt")
                        if issue == "split":
                            deng = nc.sync if n_piece % 2 == 0 else nc.scalar
                        else:
                            deng = nc.sync
                        n_piece += 1
                        deng.dma_start(
                            lt[:], latB[b * P:(b + 1) * P, k0:k0 + sz, :])
                        pieces.append(lt)
                        for i in range(sz):
                            kmap[k0 + i] = (gi, i)
                        k0 += sz
                    if drk:
                        drp = lat_pool.tile([P, 2 * drk, TPB], F8E4,
                                            name="drp", tag="lt")
                        nc.sync.dma_start(
                            drp[:], latD[b * P:(b + 1) * P, :, :])
                    do_i3 = drain == "i3all" or (
                        drain.startswith("i3last") and b == NB - 1)
                    if do_i3:
                        for j in range(bs):
                            s = b * bs + j
                            nc.tensor.matmul(
                                pss[s][:], i3_t[:],
                                nz16[:, s * SUPER:(s + 1) * SUPER],
                                start=True, stop=False)
                    for k in range(NPLAIN):
                        gi, i = kmap[k]
                        for j in range(bs):
                            s = b * bs + j
                            nc.tensor.matmul(
                                pss[s][:], wt_t[:, k * 3:(k + 1) * 3],
                                pieces[gi][:, i * TPB + j * SUPER:
                                           i * TPB + (j + 1) * SUPER],
                                start=(k == 0 and not do_i3),
                                stop=(k == NPLAIN - 1 and drk == 0),
                            )
                    for dr in range(drk):
                        for j in range(bs):
                            s = b * bs + j
                            nc.tensor.matmul(
                                pss[s][:], wt4_t[:, 2 * dr:2 * dr + 2, 0:3],
                                drp[:, 2 * dr:2 * dr + 2,
                                    j * SUPER:(j + 1) * SUPER],
                                start=False, stop=(dr == drk - 1),
                                perf_mode=mybir.MatmulPerfMode.DoubleRow,
                            )
                    for j in range(bs):
                        s = b * bs + j
                        eng = nc.sync if (j % 2 == 0 or mono) else nc.scalar
                        osb = osb_pool.tile([3, SUPER], F32,
                                            name="osb", tag="osb")
                        if do_i3:
                            if j % 2 == 0 or drain == "i3lastv" or mono:
                                nc.vector.tensor_copy(osb[:], pss[s][:])
                            else:
                                nc.scalar.copy(osb[:], pss[s][:])
                        else:
                            nc.vector.tensor_add(
                                osb[:], pss[s][:],
                                nz_t[:, s * SUPER:(s + 1) * SUPER])
                        eng.dma_start(
                            out[:, s * SUPER:(s + 1) * SUPER], osb[:])

    nc.compile()
    _NC_CACHE[key] = nc
    return nc


def _build_nc_v13(lat_dt=F8E3, wt_dt=F16, plan="1:2,1:2,2:2,4:2", bufs=8,
                  warmups=3, drain="i3last", wt_eng="scalar", repeat=1):
    """Variable block plan: ascending token-block sizes for an early first
    matmul, big blocks mid-stream, small-ish final block for a short drain.

    plan: comma list of supers:group per block; supers must sum to 8.
    Block b's DMA pieces are [128, group, supers*512] with contiguous
    per-partition runs (host packs per block).
    """
    key = ("v13", lat_dt, wt_dt, plan, bufs, warmups, drain, wt_eng, repeat)
    if key in _NC_CACHE:
        return _NC_CACHE[key]

    blocks = [(int(a), int(g)) for a, g in
              (p.split(":") for p in plan.split(","))]
    assert sum(b for b, _ in blocks) == N_SUPER

    nc = bacc.Bacc("TRN2", target_bir_lowering=False, debug=False,
                   enable_asserts=False, num_devices=N_CORES)
    latBs = [nc.dram_tensor(f"latT{bi}", [P, N_CHUNK, b * SUPER], lat_dt,
                            kind="ExternalInput").ap()
             for bi, (b, _) in enumerate(blocks)]
    wt = nc.dram_tensor("wt", [P, 3 * N_CHUNK], wt_dt, kind="ExternalInput").ap()
    nzt = nc.dram_tensor("nzt", [3, TPC], F32, kind="ExternalInput").ap()
    if drain.startswith("i3"):
        nzt16 = nc.dram_tensor("nzt16", [3, TPC], F16, kind="ExternalInput").ap()
        i3 = nc.dram_tensor("i3", [3, 3], F16, kind="ExternalInput").ap()
    out = nc.dram_tensor("out", [3, TPC], F32, kind="ExternalOutput").ap()

    with tile.TileContext(nc) as tc:
        with ExitStack() as ctx:
            const = ctx.enter_context(tc.tile_pool(name="const", bufs=1))
            lat_pool = ctx.enter_context(tc.tile_pool(name="lat", bufs=bufs))
            ps_pool = ctx.enter_context(tc.tile_pool(name="ps", bufs=1, space="PSUM"))
            osb_pool = ctx.enter_context(tc.tile_pool(name="osb", bufs=4))

            warm = const.tile([P, SUPER], wt_dt)
            nc.vector.memset(warm[:], 0.001)

            wt_t = const.tile([P, 3 * N_CHUNK], wt_dt)
            (nc.sync if wt_eng == "sync" else nc.scalar).dma_start(
                wt_t[:], wt[:])
            nz_t = const.tile([3, TPC], F32)
            nc.gpsimd.dma_start(nz_t[:], nzt[:])
            if drain.startswith("i3"):
                nz16 = const.tile([3, TPC], F16)
                nc.gpsimd.dma_start(nz16[:], nzt16[:])
                i3_t = const.tile([3, 3], F16)
                nc.scalar.dma_start(i3_t[:], i3[:])

            for _ in range(repeat):
                pss = [ps_pool.tile([3, SUPER], F32, name=f"ps{s}", tag=f"ps{s}")
                       for s in range(N_SUPER)]
                for _ in range(warmups):
                    nc.tensor.matmul(pss[N_SUPER - 1][:], warm[:, 0:3], warm[:],
                                     start=True, stop=True)
                s_base = 0
                for bi, (bsup, group) in enumerate(blocks):
                    tpb = bsup * SUPER
                    pieces = []
                    kmap = {}
                    for gi in range(N_CHUNK // group):
                        lt = lat_pool.tile([P, group * tpb], lat_dt,
                                           name="lt", tag="lt")
                        nc.sync.dma_start(
                            lt[:], latBs[bi][:, gi * group:(gi + 1) * group, :])
                        pieces.append(lt)
                        for i in range(group):
                            kmap[gi * group + i] = (gi, i)
                    do_i3 = drain == "i3all" or (
                        drain.startswith("i3last") and bi == len(blocks) - 1)
                    if do_i3:
                        for j in range(bsup):
                            s = s_base + j
                            nc.tensor.matmul(
                                pss[s][:], i3_t[:],
                                nz16[:, s * SUPER:(s + 1) * SUPER],
                                start=True, stop=False)
                    for k in range(N_CHUNK):
                        gi, i = kmap[k]
                        for j in range(bsup):
                            s = s_base + j
                            nc.tensor.matmul(
                                pss[s][:], wt_t[:, k * 3:(k + 1) * 3],
                                pieces[gi][:, i * tpb + j * SUPER:
                                           i * tpb + (j + 1) * SUPER],
                                start=(k == 0 and not do_i3),
                                stop=(k == N_CHUNK - 1),
                            )
                    for j in range(bsup):
                        s = s_base + j
                        # mid-stream stores ride GpSimd SWDGE (Sync stays
                        # dedicated to latent issue); last block's stores
                        # use the idle Sync/Scalar HWDGEs in parallel
                        if bi == len(blocks) - 1:
                            eng = nc.sync if j % 2 == 0 else nc.scalar
                        else:
                            eng = nc.gpsimd
                        osb = osb_pool.tile([3, SUPER], F32,
                                            name="osb", tag="osb")
                        if do_i3:
                            if j % 2 == 0 or drain == "i3lastv":
                                nc.vector.tensor_copy(osb[:], pss[s][:])
                            else:
                                nc.scalar.copy(osb[:], pss[s][:])
                        else:
                            nc.vector.tensor_add(
                                osb[:], pss[s][:],
                                nz_t[:, s * SUPER:(s + 1) * SUPER])
                        eng.dma_start(
                            out[:, s * SUPER:(s + 1) * SUPER], osb[:])
                    s_base += bsup

    nc.compile()
    _NC_CACHE[key] = nc
    return nc


def _coeff(T: int) -> float:
    a = 1.0
    for t in range(T):
        a *= (t + 1) / T
    return a


PIPELINE = "v11_fp8"  # "v7" | "v8_*" | "v9_*" | "v10_*" | "v11_*" | "v12_*"
_V10_GROUP = 2
_V10_BUFS = 10
_WARMUP_MMS = 6
_FINE_LEAD = False
_PAIR0_GROUP = 2
_V11_GROUP = 2
_V11_BUFS = 10
_V11_WARMUPS = 8
_V11_DRAIN = "add"    # "add" | "i3last" | "i3all"
_V11_STORE = "gpend"  # "alt" | "scalar" | "gp" | "gpend"
_V11_ISSUE = "sync"   # "sync" | "split" | "gp0"
_V11_DUMMY = 0
_V11_WTENG = "scalar"  # "scalar" | "sync" | "gp"
_V12_BS = 4
_V12_GROUP = 2
_V12_BUFS = 6
_V12_WARMUPS = 8
_V12_DRAIN = "i3last"
_V12_DUMMY = 0
_V12_LEAD = 0
_V12_WTINY = 0
_V12_WTENG = "scalar"
_V12_ISSUE = "sync"
_V12_MONO = 0
_V12_DRK = 0
_V13_PLAN = "1:2,1:2,2:2,4:2"
_V13_BUFS = 8
_V13_WARMUPS = 3
_V13_DRAIN = "i3last"
_V13_WTENG = "scalar"


def kernel(latent, W, b, noise, diffusion_steps, _trace=False, _pipeline=None):
    import ml_dtypes
    import os
    global _V11_GROUP, _V11_BUFS, _V11_WARMUPS, _V11_DRAIN, _V11_STORE
    global _V11_ISSUE, _V11_DUMMY
    _V11_GROUP = int(os.environ.get("V11_GROUP", _V11_GROUP))
    _V11_BUFS = int(os.environ.get("V11_BUFS", _V11_BUFS))
    _V11_WARMUPS = int(os.environ.get("V11_WARMUPS", _V11_WARMUPS))
    _V11_DRAIN = os.environ.get("V11_DRAIN", _V11_DRAIN)
    _V11_STORE = os.environ.get("V11_STORE", _V11_STORE)
    _V11_ISSUE = os.environ.get("V11_ISSUE", _V11_ISSUE)
    _V11_DUMMY = int(os.environ.get("V11_DUMMY", _V11_DUMMY))
    global _V11_WTENG
    _V11_WTENG = os.environ.get("V11_WTENG", _V11_WTENG)
    global _V12_BS, _V12_GROUP, _V12_BUFS, _V12_WARMUPS, _V12_DRAIN
    global _V12_DUMMY, _V12_LEAD
    _V12_BS = int(os.environ.get("V12_BS", _V12_BS))
    _V12_GROUP = int(os.environ.get("V12_GROUP", _V12_GROUP))
    _V12_BUFS = int(os.environ.get("V12_BUFS", _V12_BUFS))
    _V12_WARMUPS = int(os.environ.get("V12_WARMUPS", _V12_WARMUPS))
    _V12_DRAIN = os.environ.get("V12_DRAIN", _V12_DRAIN)
    _V12_DUMMY = int(os.environ.get("V12_DUMMY", _V12_DUMMY))
    _V12_LEAD = int(os.environ.get("V12_LEAD", _V12_LEAD))
    global _V12_WTINY, _V12_WTENG, _V12_ISSUE, _V12_MONO
    _V12_WTINY = int(os.environ.get("V12_WTINY", _V12_WTINY))
    _V12_WTENG = os.environ.get("V12_WTENG", _V12_WTENG)
    _V12_ISSUE = os.environ.get("V12_ISSUE", _V12_ISSUE)
    _V12_MONO = int(os.environ.get("V12_MONO", _V12_MONO))
    global _V12_DRK
    _V12_DRK = int(os.environ.get("V12_DRK", _V12_DRK))
    global _V13_PLAN, _V13_BUFS, _V13_WARMUPS, _V13_DRAIN, _V13_WTENG
    _V13_PLAN = os.environ.get("V13_PLAN", _V13_PLAN)
    _V13_BUFS = int(os.environ.get("V13_BUFS", _V13_BUFS))
    _V13_WARMUPS = int(os.environ.get("V13_WARMUPS", _V13_WARMUPS))
    _V13_DRAIN = os.environ.get("V13_DRAIN", _V13_DRAIN)
    _V13_WTENG = os.environ.get("V13_WTENG", _V13_WTENG)
    T = int(diffusion_steps)
    A = _coeff(T)
    pipeline = _pipeline or PIPELINE
    fp8 = pipeline.endswith("fp8")
    v9 = pipeline.startswith("v9")

    lat_flat = np.ascontiguousarray(latent.reshape(TOK, D), dtype=np.float32)
    if fp8:
        latT_h = lat_flat.astype(ml_dtypes.float8_e3m4).T  # [D, TOK] view
    else:
        latT_h = lat_flat.astype(np.float16).T
    wt_eff = np.ascontiguousarray(W.T).astype(np.float32) * np.float32(1.0 - A)
    # prepack [2048, 3] -> [128, 16*3]: chunk k (rows 128k..128k+128) at cols 3k..3k+3
    wt_packed = np.ascontiguousarray(
        wt_eff.reshape(N_CHUNK, P, 3).transpose(1, 0, 2).reshape(P, 3 * N_CHUNK)
    ).astype(np.float16)
    nz_eff = (np.float32(A) * noise.reshape(TOK, 3)
              + np.float32(1.0 - A) * b[None, :].astype(np.float32))
    nz_eff_t = np.ascontiguousarray(nz_eff.T.astype(np.float32))  # [3, TOK]

    lat_dt = mybir.dt.float8e3 if fp8 else F16
    v10 = pipeline.startswith("v10")
    v11 = pipeline.startswith("v11")
    v12 = pipeline.startswith("v12")
    v13 = pipeline.startswith("v13")
    if pipeline == "v7":
        nc = _build_nc_v7()
    elif v13:
        nc = _build_nc_v13(lat_dt=lat_dt, plan=_V13_PLAN, bufs=_V13_BUFS,
                           warmups=_V13_WARMUPS, drain=_V13_DRAIN,
                           wt_eng=_V13_WTENG)
    elif v12:
        nc = _build_nc_v12(lat_dt=lat_dt, bs=_V12_BS, group=_V12_GROUP,
                           bufs=_V12_BUFS, warmups=_V12_WARMUPS,
                           drain=_V12_DRAIN, dummy=_V12_DUMMY, lead=_V12_LEAD,
                           wtiny=_V12_WTINY, wt_eng=_V12_WTENG,
                           issue=_V12_ISSUE, mono=_V12_MONO, drk=_V12_DRK)
    elif v11:
        nc = _build_nc_v11(lat_dt=lat_dt, group=_V11_GROUP, bufs=_V11_BUFS,
                           warmups=_V11_WARMUPS, drain=_V11_DRAIN,
                           store_eng=_V11_STORE, issue=_V11_ISSUE,
                           dummy=_V11_DUMMY, wt_eng=_V11_WTENG)
    elif v10:
        nc = _build_nc_v10(lat_dt=lat_dt, group=_V10_GROUP, bufs=_V10_BUFS)
    elif v9:
        nc = _build_nc_v9(lat_dt=lat_dt)
    else:
        nc = _build_nc_v8(lat_dt=lat_dt)
    if v10:
        # [D, TOK] -> [128, 16, TOK]: partition-major chunk layout
        lat_p = np.ascontiguousarray(
            latT_h.reshape(N_CHUNK, P, TOK).transpose(1, 0, 2))
    if v11 or v12 or v13:
        # [TOK, D] fp8 -> per-core [block*128, 16, blocktok]: piece
        # (block, k-range) is contiguous per partition line in DRAM
        lat8 = latT_h.T  # the untransposed contiguous [TOK, D] downcast
        if not v13:
            blk_tok = PIECE if v11 else _V12_BS * SUPER
            n_blk = TPC // blk_tok
        drain_mode = (_V11_DRAIN if v11 else
                      _V12_DRAIN if v12 else _V13_DRAIN)
        if v13:
            v13_blocks = [int(p.split(":")[0]) for p in _V13_PLAN.split(",")]
    in_maps = []
    for c in range(N_CORES):
        if v13:
            shard = lat8[c * TPC:(c + 1) * TPC]          # [4096, 2048]
            m = {"wt": wt_packed,
                 "nzt": np.ascontiguousarray(nz_eff_t[:, c * TPC:(c + 1) * TPC])}
            t0 = 0
            for bi, bsup in enumerate(v13_blocks):
                tpb = bsup * SUPER
                sub = shard[t0:t0 + tpb]                  # [tpb, 2048]
                m[f"latT{bi}"] = np.ascontiguousarray(
                    sub.reshape(tpb, N_CHUNK, P).transpose(2, 1, 0))
                t0 += tpb
            if drain_mode.startswith("i3"):
                m["nzt16"] = m["nzt"].astype(np.float16)
                m["i3"] = np.eye(3, dtype=np.float16)
            in_maps.append(m)
            continue
        if v11 or v12:
            drk = _V12_DRK if v12 else 0
            nplain = N_CHUNK - 2 * drk
            shard = lat8[c * TPC:(c + 1) * TPC]          # [4096, 2048]
            lat_c = np.ascontiguousarray(
                shard[:, :nplain * P]
                .reshape(n_blk, blk_tok, nplain, P)
                .transpose(0, 3, 2, 1)
                .reshape(n_blk * P, nplain, blk_tok))
            m = {"latT": lat_c, "wt": wt_packed,
                 "nzt": np.ascontiguousarray(nz_eff_t[:, c * TPC:(c + 1) * TPC])}
            if drk:
                # DoubleRow chunks: e4m3 straight from the f32 source
                shard4 = lat_flat[c * TPC:(c + 1) * TPC, nplain * P:].astype(
                    ml_dtypes.float8_e4m3)
                m["latD"] = np.ascontiguousarray(
                    shard4.reshape(n_blk, blk_tok, 2 * drk, P)
                    .transpose(0, 3, 2, 1)
                    .reshape(n_blk * P, 2 * drk, blk_tok))
                wt4_h = np.zeros((P, 2 * drk, 16), dtype=ml_dtypes.float8_e4m3)
                wt4_h[:, :, 0:3] = (
                    wt_eff[nplain * P:]
                    .reshape(2 * drk, P, 3).transpose(1, 0, 2)
                    .astype(ml_dtypes.float8_e4m3))
                m["wt4"] = wt4_h
            if drain_mode.startswith("i3"):
                m["nzt16"] = m["nzt"].astype(np.float16)
                m["i3"] = np.eye(3, dtype=np.float16)
            in_maps.append(m)
            continue
        in_maps.append({
            "latT": (np.ascontiguousarray(lat_p[:, :, c * TPC:(c + 1) * TPC])
                     if v10 else
                     np.ascontiguousarray(latT_h[:, c * TPC:(c + 1) * TPC])),
            "wt": wt_packed,
            "nzt": np.ascontiguousarray(nz_eff_t[:, c * TPC:(c + 1) * TPC]),
        })
    res = run_bass_kernel_spmd(nc, in_maps, core_ids=list(range(N_CORES)),
                               trace=_trace)
    out = np.empty((TOK, 3), dtype=np.float32)
    for c in range(N_CORES):
        out[c * TPC:(c + 1) * TPC] = res.results[c]["out"].T
    if _trace:
        kernel._last_results = res
    return out.reshape(B, S, 3)

